# revision 1
# baseline (speedup 1.0000x reference)
"""
LongConvolution (causal FFT conv) Trainium2 Bass kernel.

Problem: x (4, 8192, 1024) f32, filt (1024, 8192) f32.
  y[b, l, c] = sum_m x[b, m, c] * filt[c, l - m]   (causal, per-channel)
Reference computes this via zero-padded FFT of size N = 16384.

Strategy
--------
N = 16384 = 128 * 128 -> four-step FFT where each 128-point DFT stage is a
128x128 matmul on the tensor engine.  With n = 128*n1 + n2, k = 128*k2 + k1:

  A[k1,n2]  = sum_n1 w128^(n1 k1) u[128 n1 + n2]          (matmul vs DFT-128)
  B         = A * T,  T[k1,n2] = wN^(n2 k1)               (twiddle, DVE)
  R[k1,k2]  = sum_n2 B[k1,n2] w128^(n2 k2)                (matmul)
  R^T[k2,k1] = FFT_N(u_pad)[128 k2 + k1]  -> scrambled layout = fft().reshape!
  P = R * K  (filter spectrum K precomputed on HOST in the same layout)
  inverse: mirror image with conj twiddles; only first 64 output rows needed.

Stationary operands alternate between data (F1, I1) and constant DFT matrices
(F2, I2), which makes every stage's input layout exactly what the previous
stage produced - zero on-chip transposes.

Sharding: d_model across the 8 cores (128 channels each); each core handles
all 4 batches of its channels (filter spectrum reused across batch).

Host pre/post: x is transposed per-core to (c, b, l) so every DMA is >=512B
contiguous; output comes back as (c, b, l) and is transposed into (b, l, c).
"""

import os
import sys

import numpy as np

for p in ("/opt/trn_rl_repo",):
    if p not in sys.path:
        sys.path.insert(0, p)

os.environ.setdefault("MYCRO_LOCAL_CACHE", "1")

# ----------------------------------------------------------------------------
# configuration
# ----------------------------------------------------------------------------
B, L, D = 4, 8192, 1024
NFFT = 2 * L               # 16384 = 128 * 128
NC = 8                     # cores
CPC = D // NC              # channels per core = 128

# dtype config: "f32" (exact, slow) or fast variants
MM_DT = os.environ.get("LC_MM_DT", "f32")   # F1 matmul family: f32 | f32r
TT_DT = os.environ.get("LC_TT_DT", "f32")   # elementwise + F2/I1/I2: f32 | f16 | bf16


def _consts():
    """DFT / twiddle constant matrices (float64 -> cast at use site)."""
    j = np.arange(128)
    ang128 = 2 * np.pi * np.outer(j, j) / 128
    angN = 2 * np.pi * np.outer(j, j) / NFFT
    c = {}
    c["F_cos"] = np.cos(ang128)
    c["F_sin"] = np.sin(ang128)
    c["Tw_cos"] = np.cos(angN)
    c["Tw_sin"] = np.sin(angN)
    return c


def _build_program():
    import concourse.bacc as bacc
    import concourse.bass as bass
    import concourse.mybir as mybir
    from concourse import tile

    f32 = mybir.dt.float32
    dt_mm = {"f32": mybir.dt.float32, "f32r": mybir.dt.float32r}[MM_DT]
    dt_tt = {
        "f32": mybir.dt.float32,
        "f16": mybir.dt.float16,
        "bf16": mybir.dt.bfloat16,
    }[TT_DT]
    cast_tt = TT_DT != "f32"

    nc = bacc.Bacc(None, target_bir_lowering=False, debug=False)

    # --- DRAM I/O ---
    xw = nc.dram_tensor("xw", (CPC, B, L), dt_mm, kind="ExternalInput")
    kfre = nc.dram_tensor("kfre", (CPC, 128, 128), dt_tt, kind="ExternalInput")
    kfim = nc.dram_tensor("kfim", (CPC, 128, 128), dt_tt, kind="ExternalInput")
    f1mov_d = nc.dram_tensor("f1mov", (128, 256), dt_mm, kind="ExternalInput")
    f2re_d = nc.dram_tensor("f2re", (128, 128), dt_tt, kind="ExternalInput")
    f2im_d = nc.dram_tensor("f2im", (128, 128), dt_tt, kind="ExternalInput")
    f2sin_d = nc.dram_tensor("f2sin", (128, 128), dt_tt, kind="ExternalInput")
    fcmov_d = nc.dram_tensor("fcmov", (128, 384), dt_tt, kind="ExternalInput")
    gre_d = nc.dram_tensor("gre", (128, 64), dt_tt, kind="ExternalInput")
    gimn_d = nc.dram_tensor("gimn", (128, 64), dt_tt, kind="ExternalInput")
    t1re_d = nc.dram_tensor("t1re2", (128, 128), dt_tt, kind="ExternalInput")
    t1im_d = nc.dram_tensor("t1im2", (128, 128), dt_tt, kind="ExternalInput")
    yw = nc.dram_tensor("yw", (CPC, B, L), f32, kind="ExternalOutput")

    G = B  # all 4 batch signals of a channel processed as one group

    with tile.TileContext(nc) as tc:
        with (
            tc.tile_pool(name="const", bufs=1) as constp,
            tc.tile_pool(name="kf", bufs=4) as kfp,
            tc.tile_pool(name="m", bufs=4) as mp,
            tc.tile_pool(name="work", bufs=4) as wp,
            tc.tile_pool(name="out", bufs=4) as op,
            tc.tile_pool(name="pa", bufs=2, space="PSUM") as pap,
            tc.tile_pool(name="pr", bufs=2, space="PSUM") as prp,
            tc.tile_pool(name="pc", bufs=2, space="PSUM") as pcp,
            tc.tile_pool(name="py", bufs=2, space="PSUM") as pyp,
        ):
            # constants, DMA'd once
            f1mov = constp.tile([128, 256], dt_mm)
            f2re = constp.tile([128, 128], dt_tt)
            f2im = constp.tile([128, 128], dt_tt)
            f2sin = constp.tile([128, 128], dt_tt)
            fcmov = constp.tile([128, 384], dt_tt)
            gre = constp.tile([128, 64], dt_tt)
            gimn = constp.tile([128, 64], dt_tt)
            t1re = constp.tile([128, 128], dt_tt)
            t1im = constp.tile([128, 128], dt_tt)
            nc.sync.dma_start(f1mov[:], f1mov_d[:])
            nc.sync.dma_start(f2re[:], f2re_d[:])
            nc.sync.dma_start(f2im[:], f2im_d[:])
            nc.sync.dma_start(f2sin[:], f2sin_d[:])
            nc.sync.dma_start(fcmov[:], fcmov_d[:])
            nc.sync.dma_start(gre[:], gre_d[:])
            nc.sync.dma_start(gimn[:], gimn_d[:])
            nc.sync.dma_start(t1re[:], t1re_d[:])
            nc.sync.dma_start(t1im[:], t1im_d[:])
            t1re_b = t1re[:].rearrange("p (s n) -> p s n", s=1).broadcast_to([128, G, 128])
            t1im_b = t1im[:].rearrange("p (s n) -> p s n", s=1).broadcast_to([128, G, 128])

            for c in range(CPC):
                kre = kfp.tile([128, 128], dt_tt, tag="kre")
                kim = kfp.tile([128, 128], dt_tt, tag="kim")
                nc.sync.dma_start(kre[:], kfre[c][:])
                nc.sync.dma_start(kim[:], kfim[c][:])
                kre_b = kre[:].rearrange("p (s n) -> p s n", s=1).broadcast_to([128, G, 128])
                kim_b = kim[:].rearrange("p (s n) -> p s n", s=1).broadcast_to([128, G, 128])

                # F1: A^T = M^T @ [F_re | F_im-],  K = 64 (upper half zero)
                m4 = mp.tile([64, G, 128], dt_mm, tag="m")
                nc.sync.dma_start(
                    m4[:], xw[c].rearrange("b (a n) -> a b n", n=128)
                )
                asrc = wp.tile([128, G, 256], dt_tt, tag="asb")
                for g in range(G // 2):
                    pa = pap.tile([128, 2, 256], f32, tag="pa")
                    for i in range(2):
                        j = 2 * g + i
                        nc.tensor.matmul(
                            pa[:, i, :], m4[:, j, :], f1mov[0:64, :],
                            start=True, stop=True,
                        )
                    nc.scalar.copy(
                        out=asrc[:, 2 * g : 2 * g + 2, :], in_=pa[:]
                    )

                # T1 twiddle: B = A * T1
                a_re = asrc[:, :, 0:128]
                a_im = asrc[:, :, 128:256]
                u1 = wp.tile([128, G, 128], dt_tt, tag="u1")
                u2 = wp.tile([128, G, 128], dt_tt, tag="u2")
                u3 = wp.tile([128, G, 128], dt_tt, tag="u3")
                u4 = wp.tile([128, G, 128], dt_tt, tag="u4")
                b_t = wp.tile([128, G, 256], dt_tt, tag="b")
                nc.vector.tensor_mul(u1[:], a_re, t1re_b)
                nc.vector.tensor_mul(u2[:], a_im, t1im_b)
                nc.vector.tensor_sub(b_t[:, :, 0:128], u1[:], u2[:])
                nc.vector.tensor_mul(u3[:], a_re, t1im_b)
                nc.vector.tensor_mul(u4[:], a_im, t1re_b)
                nc.vector.tensor_add(b_t[:, :, 128:256], u3[:], u4[:])

                # F2: R^T = F- @ B^T  [k2, k1]; sign of the sin-part lives in
                # the constants (f2sin / f2im), so no negated-B tile is needed
                rsrc = wp.tile([128, G, 256], dt_tt, tag="rsb")
                for g in range(G // 2):  # one psum bank per 2 signals
                    sl = slice(2 * g, 2 * g + 2)
                    b_re = b_t[:, sl, 0:128]
                    b_im = b_t[:, sl, 128:256]
                    pr = prp.tile([128, 2, 256], f32, tag="pr")
                    nc.tensor.matmul(
                        pr[:, :, 0:128], f2re[:], b_re, start=True, stop=False
                    )
                    nc.tensor.matmul(
                        pr[:, :, 0:128], f2sin[:], b_im, start=False, stop=True
                    )
                    nc.tensor.matmul(
                        pr[:, :, 128:256], f2re[:], b_im, start=True, stop=False
                    )
                    nc.tensor.matmul(
                        pr[:, :, 128:256], f2im[:], b_re, start=False, stop=True
                    )
                    nc.scalar.copy(out=rsrc[:, sl, :], in_=pr[:])

                # pointwise with filter spectrum: P = R * K
                r_re = rsrc[:, :, 0:128]
                r_im = rsrc[:, :, 128:256]
                v1 = wp.tile([128, G, 128], dt_tt, tag="u1")
                v2 = wp.tile([128, G, 128], dt_tt, tag="u2")
                v3 = wp.tile([128, G, 128], dt_tt, tag="u3")
                v4 = wp.tile([128, G, 128], dt_tt, tag="u4")
                p_re = wp.tile([128, G, 128], dt_tt, tag="pre")
                p_im = wp.tile([128, G, 128], dt_tt, tag="pim")
                nc.vector.tensor_mul(v1[:], r_re, kre_b)
                nc.vector.tensor_mul(v2[:], r_im, kim_b)
                nc.vector.tensor_sub(p_re[:], v1[:], v2[:])
                nc.vector.tensor_mul(v3[:], r_re, kim_b)
                nc.vector.tensor_mul(v4[:], r_im, kre_b)
                nc.vector.tensor_add(p_im[:], v3[:], v4[:])

                # I1: C = P @ F+   [k1, n2]  (data-stationary)
                csrc = wp.tile([128, G, 256], dt_tt, tag="csb")
                for g in range(G // 2):
                    pc = pcp.tile([128, 2, 256], f32, tag="pc")
                    for i in range(2):
                        j = 2 * g + i
                        nc.tensor.matmul(
                            pc[:, i, :], p_re[:, j, :], fcmov[:, 128:384],
                            start=True, stop=False,
                        )
                        nc.tensor.matmul(
                            pc[:, i, :], p_im[:, j, :], fcmov[:, 0:256],
                            start=False, stop=True,
                        )
                    nc.scalar.copy(
                        out=csrc[:, 2 * g : 2 * g + 2, :], in_=pc[:]
                    )

                # T2 twiddle: C' = C * conj(T1)
                c_re = csrc[:, :, 0:128]
                c_im = csrc[:, :, 128:256]
                w1 = wp.tile([128, G, 128], dt_tt, tag="u1")
                w2 = wp.tile([128, G, 128], dt_tt, tag="u2")
                w3 = wp.tile([128, G, 128], dt_tt, tag="u3")
                w4 = wp.tile([128, G, 128], dt_tt, tag="u4")
                cp_re = wp.tile([128, G, 128], dt_tt, tag="cpre")
                cp_im = wp.tile([128, G, 128], dt_tt, tag="cpim")
                nc.vector.tensor_mul(w1[:], c_re, t1re_b)
                nc.vector.tensor_mul(w2[:], c_im, t1im_b)
                nc.vector.tensor_add(cp_re[:], w1[:], w2[:])
                nc.vector.tensor_mul(w3[:], c_re, t1im_b)
                nc.vector.tensor_mul(w4[:], c_im, t1re_b)
                nc.vector.tensor_sub(cp_im[:], w4[:], w3[:])

                # I2: y = Re(F+ @ C'), first 64 rows; 1/N folded into K
                ysb = op.tile([64, G, 128], f32, tag="ysb")
                for g in range(G // 2):
                    sl = slice(2 * g, 2 * g + 2)
                    py = pyp.tile([64, 2, 128], f32, tag="py")
                    nc.tensor.matmul(
                        py[:], gre[:], cp_re[:, sl, :], start=True, stop=False
                    )
                    nc.tensor.matmul(
                        py[:], gimn[:], cp_im[:, sl, :], start=False, stop=True
                    )
                    nc.scalar.copy(out=ysb[:, sl, :], in_=py[:])
                nc.sync.dma_start(
                    yw[c].rearrange("b (a n) -> a b n", n=128), ysb[:]
                )

    nc.compile()
    return nc


def _host_arrays():
    cst = _consts()
    F_cos, F_sin = cst["F_cos"], cst["F_sin"]
    Tw_cos, Tw_sin = cst["Tw_cos"], cst["Tw_sin"]

    np_tt = {"f32": np.float32, "f16": np.float16, "bf16": None}[TT_DT]
    if np_tt is None:
        import ml_dtypes

        np_tt = ml_dtypes.bfloat16
    np_mm = np.float32

    arrs = {}
    arrs["f1mov"] = np.concatenate([F_cos, -F_sin], axis=1).astype(np_mm)
    arrs["f2re"] = F_cos.astype(np_tt)
    arrs["f2im"] = (-F_sin).astype(np_tt)
    arrs["f2sin"] = F_sin.astype(np_tt)
    # fcmov = [F+_im_neg | F+_re | F+_im] = [-sin | cos | sin]
    arrs["fcmov"] = np.concatenate([-F_sin, F_cos, F_sin], axis=1).astype(np_tt)
    # 1/NFFT normalization lives in the host-side filter spectrum (keeps
    # every on-chip intermediate within fp16 range)
    arrs["gre"] = F_cos[:, :64].astype(np_tt)
    arrs["gimn"] = (-F_sin[:, :64]).astype(np_tt)
    arrs["t1re2"] = Tw_cos.astype(np_tt)
    arrs["t1im2"] = (-Tw_sin).astype(np_tt)
    return arrs, np_tt


def kernel(x: np.ndarray, filt: np.ndarray) -> np.ndarray:
    from concourse.bass_utils import run_bass_kernel_spmd

    assert x.shape == (B, L, D) and filt.shape == (D, L)
    x = np.ascontiguousarray(x, dtype=np.float32)
    filt = np.ascontiguousarray(filt, dtype=np.float32)

    consts, np_tt = _host_arrays()

    # filter spectrum: FFT of zero-padded filter; reshape(128,128) IS the
    # scrambled [k2,k1] layout produced by the on-device four-step forward.
    kpad = np.zeros((D, NFFT), np.float64)
    kpad[:, :L] = filt
    Kf = (np.fft.fft(kpad, axis=1) / NFFT).reshape(D, 128, 128)

    in_maps = []
    for ci in range(NC):
        sl = slice(ci * CPC, (ci + 1) * CPC)
        m = dict(consts)
        m["xw"] = np.ascontiguousarray(x[:, :, sl].transpose(2, 0, 1))
        m["kfre"] = np.ascontiguousarray(Kf[sl].real.astype(np_tt))
        m["kfim"] = np.ascontiguousarray(Kf[sl].imag.astype(np_tt))
        in_maps.append(m)

    nc = _build_program()
    res = run_bass_kernel_spmd(nc, in_maps, core_ids=list(range(NC)))

    y = np.empty((B, L, D), np.float32)
    for ci in range(NC):
        sl = slice(ci * CPC, (ci + 1) * CPC)
        y[:, :, sl] = res.results[ci]["yw"].transpose(1, 2, 0)
    return y


def run_profiled(inputs):
    """Build + run with NTFF tracing; returns BassKernelResults (test-only)."""
    from concourse.bass_utils import run_bass_kernel_spmd

    x = np.ascontiguousarray(inputs["x"], dtype=np.float32)
    filt = np.ascontiguousarray(inputs["filt"], dtype=np.float32)
    consts, np_tt = _host_arrays()
    kpad = np.zeros((D, NFFT), np.float64)
    kpad[:, :L] = filt
    Kf = (np.fft.fft(kpad, axis=1) / NFFT).reshape(D, 128, 128)
    in_maps = []
    for ci in range(NC):
        sl = slice(ci * CPC, (ci + 1) * CPC)
        m = dict(consts)
        m["xw"] = np.ascontiguousarray(x[:, :, sl].transpose(2, 0, 1))
        m["kfre"] = np.ascontiguousarray(Kf[sl].real.astype(np_tt))
        m["kfim"] = np.ascontiguousarray(Kf[sl].imag.astype(np_tt))
        in_maps.append(m)
    nc = _build_program()
    return run_bass_kernel_spmd(
        nc, in_maps, core_ids=list(range(NC)), trace=True
    )


if __name__ == "__main__":
    rng = np.random.default_rng(0)
    x = rng.standard_normal((B, L, D)).astype(np.float32)
    filt = rng.standard_normal((D, L)).astype(np.float32)
    y = kernel(x, filt)
    print("y", y.shape, y.dtype, float(np.abs(y).max()))



# revision 5
# speedup vs baseline: 3.2982x; 3.2982x over previous
"""
LongConvolution (causal FFT conv) Trainium2 Bass kernel — v2.

Problem: x (4, 8192, 1024) f32, filt (1024, 8192) f32.
  y[b, l, c] = sum_m x[b, m, c] * filt[c, l - m]   (causal, per-channel)
Reference computes this via zero-padded FFT of size N = 16384 = 128*128.

v2 strategy (vs v1: ~1.45 ms, fp32 four-step, 4 real pipelines)
---------------------------------------------------------------
1. Packed-complex batches: z = x[2p] + i*x[2p+1].  The filter multiply is
   linear, so IFFT(FFT(z) * K) = y[2p] + i*y[2p+1] with NO Hermitian
   unpacking.  4 real convs become 2 complex pipelines: halves F2/I1
   matmul work and ALL twiddle/pointwise elementwise work.
2. f16 everywhere on-chip: matmuls at 1 cycle/row (4x vs f32), DVE
   elementwise at 2x rate.  PSUM stays f32 (hardware).
3. 4-channel iterations: elementwise ops run at free-size 1024 to
   amortize fixed per-op overheads; PSUM tiles stay per-channel (1 bank)
   so all four stages double-buffer in the 8 banks.
4. Engine balance: PE matmuls ~218us, DVE 15 elementwise ops, Pool
   (gpsimd) 3 elementwise ops + 2 output copies, ACT 12 PSUM->SBUF
   converting copies + 2 output copies.

Four-step FFT (k = 128*k2 + k1, n = 128*n1 + n2):
  A[n2,k1] = sum_n1 e^{-2pi i n1 k1/128} z[128 n1 + n2]     (F1, K=64)
  B = A * T1,  T1[n2,k1] = e^{-2pi i n2 k1/N}               (T1, DVE)
  R[k2,k1] = sum_n2 e^{-2pi i n2 k2/128} B[n2,k1]           (F2)
  P = R * K   (filter spectrum, host-precomputed, [k2,k1])  (PW)
  C[k1,n2] = sum_k2 e^{+2pi i n2 k2/128} P[k2,k1]           (I1)
  C' = C * conj(T1)  ([k1,n2] layout, T1 symmetric)         (T2)
  y[n1,n2] = sum_k1 e^{+2pi i n1 k1/128} C'[k1,n2], n1<64   (I2)
  y[2p] = Re, y[2p+1] = Im.

Sharding: d_model across the 8 cores (128 channels each).
"""

import os
import sys

import numpy as np

for p in ("/opt/trn_rl_repo",):
    if p not in sys.path:
        sys.path.insert(0, p)

os.environ.setdefault("MYCRO_LOCAL_CACHE", "1")

# ----------------------------------------------------------------------------
# configuration
# ----------------------------------------------------------------------------
B, L, D = 4, 8192, 1024
NFFT = 2 * L               # 16384 = 128 * 128
NC = 8                     # cores
CPC = D // NC              # channels per core = 128
CHG = 4                    # channels per iteration
NIT = CPC // CHG           # 32 iterations


def _consts():
    j = np.arange(128)
    ang128 = 2 * np.pi * np.outer(j, j) / 128
    angN = 2 * np.pi * np.outer(j, j) / NFFT
    return {
        "F_cos": np.cos(ang128), "F_sin": np.sin(ang128),
        "Tw_cos": np.cos(angN), "Tw_sin": np.sin(angN),
    }


def _build_program():
    import concourse.bacc as bacc
    import concourse.mybir as mybir
    from concourse import tile

    f32 = mybir.dt.float32
    f16 = mybir.dt.float16

    nc = bacc.Bacc(None, target_bir_lowering=False, debug=False)

    # --- DRAM I/O (all f16 except noted) ---
    # xw[it, n1, 2*chl+pk, ri*128+n2]
    xw = nc.dram_tensor("xw", (NIT, 64, 2 * CHG, 256), f16, kind="ExternalInput")
    # kk[it, k2, chl, ri, k1]
    kk = nc.dram_tensor("kk", (NIT, 128, CHG, 2, 128), f16, kind="ExternalInput")
    # constants
    f1m_d = nc.dram_tensor("f1m", (64, 512), f16, kind="ExternalInput")
    f2c_d = nc.dram_tensor("f2c", (128, 128), f16, kind="ExternalInput")
    f2s_d = nc.dram_tensor("f2s", (128, 128), f16, kind="ExternalInput")
    f2sn_d = nc.dram_tensor("f2sn", (128, 128), f16, kind="ExternalInput")
    fim_d = nc.dram_tensor("fim", (128, 512), f16, kind="ExternalInput")
    gc_d = nc.dram_tensor("gc", (128, 64), f16, kind="ExternalInput")
    gs_d = nc.dram_tensor("gs", (128, 64), f16, kind="ExternalInput")
    gsn_d = nc.dram_tensor("gsn", (128, 64), f16, kind="ExternalInput")
    t1re_d = nc.dram_tensor("t1re", (128, 128), f16, kind="ExternalInput")
    t1im_d = nc.dram_tensor("t1im", (128, 128), f16, kind="ExternalInput")
    # yw[pair, cl, n1, pk, sig, n2] — two channels share one PSUM bank
    # (partition halves), so the partition dim of the out DMA is (cl, n1)
    yw = nc.dram_tensor(
        "yw", (CPC // 2, 2, 64, 2, 2, 128), f16, kind="ExternalOutput"
    )

    with tile.TileContext(nc) as tc:
        with (
            tc.tile_pool(name="const", bufs=1) as constp,
            tc.tile_pool(name="m", bufs=2) as mp,
            tc.tile_pool(name="kf", bufs=2) as kp,
            tc.tile_pool(name="work", bufs=3) as wp,
            tc.tile_pool(name="scr", bufs=4) as up,
            tc.tile_pool(name="out", bufs=3) as op,
            tc.tile_pool(name="pa", bufs=2, space="PSUM") as pap,
            tc.tile_pool(name="pr", bufs=2, space="PSUM") as prp,
            tc.tile_pool(name="pc", bufs=2, space="PSUM") as pcp,
            tc.tile_pool(name="py", bufs=2, space="PSUM") as pyp,
        ):
            f1m = constp.tile([64, 512], f16)
            f2c = constp.tile([128, 128], f16)
            f2s = constp.tile([128, 128], f16)
            f2sn = constp.tile([128, 128], f16)
            fim = constp.tile([128, 512], f16)
            gc = constp.tile([128, 64], f16)
            gs = constp.tile([128, 64], f16)
            gsn = constp.tile([128, 64], f16)
            t1re = constp.tile([128, 128], f16)
            t1im = constp.tile([128, 128], f16)
            for t, d in (
                (f1m, f1m_d), (f2c, f2c_d), (f2s, f2s_d), (f2sn, f2sn_d),
                (fim, fim_d), (gc, gc_d), (gs, gs_d), (gsn, gsn_d),
                (t1re, t1re_d), (t1im, t1im_d),
            ):
                nc.sync.dma_start(t[:], d[:])
            t1re_b = (
                t1re[:].rearrange("p (s n) -> p s n", s=1)
                .broadcast_to([128, 2 * CHG, 128])
            )
            t1im_b = (
                t1im[:].rearrange("p (s n) -> p s n", s=1)
                .broadcast_to([128, 2 * CHG, 128])
            )

            for it in range(NIT):
                m4 = mp.tile([64, 2 * CHG, 256], f16, tag="m")
                nc.sync.dma_start(m4[:], xw[it])
                kt = kp.tile([128, CHG, 2, 128], f16, tag="k")
                nc.sync.dma_start(kt[:], kk[it])

                # ---- F1 + copy: pa -> asrc ----
                asrc = wp.tile([128, 2, 2 * CHG, 128], f16, tag="asrc")
                for ch in range(CHG):
                    pa = pap.tile([128, 2, 2, 128], f32, tag="pa")
                    for pk in range(2):
                        j = 2 * ch + pk
                        nc.tensor.matmul(
                            pa[:, :, pk, :], m4[:, j, 0:128], f1m[:, 0:256],
                            start=True, stop=False,
                        )
                        nc.tensor.matmul(
                            pa[:, :, pk, :], m4[:, j, 128:256], f1m[:, 256:512],
                            start=False, stop=True,
                        )
                    nc.scalar.copy(
                        out=asrc[:, :, 2 * ch : 2 * ch + 2, :], in_=pa[:]
                    )

                # ---- T1 twiddle (DVE + Pool) ----
                a_re = asrc[:, 0, :, :]
                a_im = asrc[:, 1, :, :]
                b_t = wp.tile([128, 2, 2 * CHG, 128], f16, tag="b")
                u1 = up.tile([128, 2 * CHG, 128], f16, tag="u1")
                u2 = up.tile([128, 2 * CHG, 128], f16, tag="u2")
                u3 = up.tile([128, 2 * CHG, 128], f16, tag="u3")
                u4 = up.tile([128, 2 * CHG, 128], f16, tag="u4")
                nc.vector.tensor_mul(u1[:], a_re, t1re_b)
                nc.gpsimd.tensor_mul(u2[:], a_im, t1im_b)
                nc.vector.tensor_mul(u3[:], a_re, t1im_b)
                nc.vector.tensor_mul(u4[:], a_im, t1re_b)
                nc.vector.tensor_sub(b_t[:, 0, :, :], u1[:], u2[:])
                nc.vector.tensor_add(b_t[:, 1, :, :], u3[:], u4[:])

                # ---- F2 + copy: pr -> rsrc ----
                rsrc = wp.tile([128, 2, 2 * CHG, 128], f16, tag="rsrc")
                for ch in range(CHG):
                    pr = prp.tile([128, 2, 2, 128], f32, tag="pr")
                    for pk in range(2):
                        j = 2 * ch + pk
                        b_re = b_t[:, 0, j, :]
                        b_im = b_t[:, 1, j, :]
                        nc.tensor.matmul(
                            pr[:, 0, pk, :], f2c[:], b_re, start=True, stop=False
                        )
                        nc.tensor.matmul(
                            pr[:, 0, pk, :], f2s[:], b_im, start=False, stop=True
                        )
                        nc.tensor.matmul(
                            pr[:, 1, pk, :], f2c[:], b_im, start=True, stop=False
                        )
                        nc.tensor.matmul(
                            pr[:, 1, pk, :], f2sn[:], b_re, start=False, stop=True
                        )
                    nc.scalar.copy(
                        out=rsrc[:, :, 2 * ch : 2 * ch + 2, :], in_=pr[:]
                    )

                # ---- pointwise filter multiply (DVE) ----
                r_re = rsrc[:, 0, :, :].rearrange("p (c k) n -> p c k n", c=CHG)
                r_im = rsrc[:, 1, :, :].rearrange("p (c k) n -> p c k n", c=CHG)
                kre_b = (
                    kt[:, :, 0, :].rearrange("p c (s n) -> p c s n", s=1)
                    .broadcast_to([128, CHG, 2, 128])
                )
                kim_b = (
                    kt[:, :, 1, :].rearrange("p c (s n) -> p c s n", s=1)
                    .broadcast_to([128, CHG, 2, 128])
                )
                p_t = wp.tile([128, 2, 2 * CHG, 128], f16, tag="p")
                p_re = p_t[:, 0, :, :].rearrange("p (c k) n -> p c k n", c=CHG)
                p_im = p_t[:, 1, :, :].rearrange("p (c k) n -> p c k n", c=CHG)
                v1 = up.tile([128, CHG, 2, 128], f16, tag="u1")
                v2 = up.tile([128, CHG, 2, 128], f16, tag="u2")
                v3 = up.tile([128, CHG, 2, 128], f16, tag="u3")
                v4 = up.tile([128, CHG, 2, 128], f16, tag="u4")
                nc.vector.tensor_mul(v1[:], r_re, kre_b)
                nc.vector.tensor_mul(v2[:], r_im, kim_b)
                nc.vector.tensor_sub(p_re, v1[:], v2[:])
                nc.vector.tensor_mul(v3[:], r_re, kim_b)
                nc.vector.tensor_mul(v4[:], r_im, kre_b)
                nc.vector.tensor_add(p_im, v3[:], v4[:])

                # ---- I1 + copy: pc -> csrc ----
                csrc = wp.tile([128, 2, 2 * CHG, 128], f16, tag="csrc")
                for ch in range(CHG):
                    pc = pcp.tile([128, 2, 2, 128], f32, tag="pc")
                    for pk in range(2):
                        j = 2 * ch + pk
                        nc.tensor.matmul(
                            pc[:, :, pk, :], p_t[:, 0, j, :], fim[:, 0:256],
                            start=True, stop=False,
                        )
                        nc.tensor.matmul(
                            pc[:, :, pk, :], p_t[:, 1, j, :], fim[:, 256:512],
                            start=False, stop=True,
                        )
                    nc.scalar.copy(
                        out=csrc[:, :, 2 * ch : 2 * ch + 2, :], in_=pc[:]
                    )

                # ---- T2 twiddle (DVE + Pool) ----
                c_re = csrc[:, 0, :, :]
                c_im = csrc[:, 1, :, :]
                cp_t = wp.tile([128, 2, 2 * CHG, 128], f16, tag="cp")
                w1 = up.tile([128, 2 * CHG, 128], f16, tag="u1")
                w2 = up.tile([128, 2 * CHG, 128], f16, tag="u2")
                w3 = up.tile([128, 2 * CHG, 128], f16, tag="u3")
                w4 = up.tile([128, 2 * CHG, 128], f16, tag="u4")
                nc.vector.tensor_mul(w1[:], c_re, t1re_b)
                nc.gpsimd.tensor_mul(w2[:], c_im, t1im_b)
                nc.vector.tensor_mul(w3[:], c_re, t1im_b)
                nc.gpsimd.tensor_mul(w4[:], c_im, t1re_b)
                nc.vector.tensor_add(cp_t[:, 0, :, :], w1[:], w2[:])
                nc.vector.tensor_sub(cp_t[:, 1, :, :], w4[:], w3[:])

                # ---- I2 + copy + DMA out (2 channels per PSUM bank) ----
                for ch in range(CHG):
                    if ch % 2 == 0:
                        py = pyp.tile([128, 2, 2, 128], f32, tag="py")
                    base = 64 * (ch % 2)
                    for pk in range(2):
                        j = 2 * ch + pk
                        cp_re = cp_t[:, 0, j, :]
                        cp_im = cp_t[:, 1, j, :]
                        nc.tensor.matmul(
                            py[base : base + 64, pk, 0, :], gc[:], cp_re,
                            start=True, stop=False,
                        )
                        nc.tensor.matmul(
                            py[base : base + 64, pk, 0, :], gsn[:], cp_im,
                            start=False, stop=True,
                        )
                        nc.tensor.matmul(
                            py[base : base + 64, pk, 1, :], gs[:], cp_re,
                            start=True, stop=False,
                        )
                        nc.tensor.matmul(
                            py[base : base + 64, pk, 1, :], gc[:], cp_im,
                            start=False, stop=True,
                        )
                    if ch % 2 == 1:
                        pair = (CHG * it + ch) // 2
                        ysb = op.tile([128, 2, 2, 128], f16, tag="ysb")
                        nc.scalar.copy(out=ysb[:], in_=py[:])
                        nc.sync.dma_start(
                            yw[pair].rearrange("c a k s n -> (c a) k s n"),
                            ysb[:],
                        )

    nc.compile()
    return nc


def _host_arrays():
    cst = _consts()
    F_cos, F_sin = cst["F_cos"], cst["F_sin"]
    Tw_cos, Tw_sin = cst["Tw_cos"], cst["Tw_sin"]
    f16 = np.float16
    cosF, sinF = F_cos[:64, :], F_sin[:64, :]
    arrs = {}
    arrs["f1m"] = np.concatenate(
        [cosF, -sinF, sinF, cosF], axis=1
    ).astype(f16)
    arrs["f2c"] = F_cos.astype(f16)
    arrs["f2s"] = F_sin.astype(f16)
    arrs["f2sn"] = (-F_sin).astype(f16)
    arrs["fim"] = np.concatenate(
        [F_cos, F_sin, -F_sin, F_cos], axis=1
    ).astype(f16)
    arrs["gc"] = F_cos[:, :64].astype(f16)
    arrs["gs"] = F_sin[:, :64].astype(f16)
    arrs["gsn"] = (-F_sin[:, :64]).astype(f16)
    arrs["t1re"] = Tw_cos.astype(f16)
    arrs["t1im"] = (-Tw_sin).astype(f16)
    return arrs


def _prep_inputs(x, filt):
    """Full inputs -> list of per-core input maps."""
    consts = _host_arrays()

    kpad = np.zeros((D, NFFT), np.float64)
    kpad[:, :L] = filt
    Kf = (np.fft.fft(kpad, axis=1) / NFFT).reshape(D, 128, 128)  # [c, k2, k1]

    # x -> (D, 2pk, 2ri, 64 n1, 128 n2)
    xq = np.ascontiguousarray(x.transpose(2, 0, 1)).reshape(D, 2, 2, 64, 128)

    in_maps = []
    for ci in range(NC):
        sl = slice(ci * CPC, (ci + 1) * CPC)
        m = dict(consts)
        xc = xq[sl].reshape(NIT, CHG, 2, 2, 64, 128)
        # -> (it, n1, chl, pk, ri*n2)
        m["xw"] = np.ascontiguousarray(
            xc.transpose(0, 4, 1, 2, 3, 5).reshape(NIT, 64, 2 * CHG, 256)
        ).astype(np.float16)
        kc = Kf[sl]
        kri = np.stack([kc.real, kc.imag], axis=1)  # (CPC, 2ri, 128k2, 128k1)
        m["kk"] = np.ascontiguousarray(
            kri.reshape(NIT, CHG, 2, 128, 128).transpose(0, 3, 1, 2, 4)
        ).astype(np.float16)
        in_maps.append(m)
    return in_maps


def _post_outputs(res):
    y = np.empty((B, L, D), np.float32)
    for ci in range(NC):
        sl = slice(ci * CPC, (ci + 1) * CPC)
        # (pair, cl, n1, pk, sig, n2); c = 2*pair+cl, b = 2*pk+sig,
        # l = 128*n1+n2
        r = res.results[ci]["yw"].astype(np.float32)
        r = r.transpose(3, 4, 2, 5, 0, 1).reshape(B, L, CPC)
        y[:, :, sl] = r
    return y


def kernel(x: np.ndarray, filt: np.ndarray) -> np.ndarray:
    from concourse.bass_utils import run_bass_kernel_spmd

    assert x.shape == (B, L, D) and filt.shape == (D, L)
    x = np.ascontiguousarray(x, dtype=np.float32)
    filt = np.ascontiguousarray(filt, dtype=np.float32)

    in_maps = _prep_inputs(x, filt)
    nc = _build_program()
    res = run_bass_kernel_spmd(nc, in_maps, core_ids=list(range(NC)))
    return _post_outputs(res)


def run_profiled(inputs):
    """Build + run with NTFF tracing; returns BassKernelResults (test-only)."""
    from concourse.bass_utils import run_bass_kernel_spmd

    x = np.ascontiguousarray(inputs["x"], dtype=np.float32)
    filt = np.ascontiguousarray(inputs["filt"], dtype=np.float32)
    in_maps = _prep_inputs(x, filt)
    nc = _build_program()
    return run_bass_kernel_spmd(
        nc, in_maps, core_ids=list(range(NC)), trace=True
    )


if __name__ == "__main__":
    rng = np.random.default_rng(0)
    x = rng.standard_normal((B, L, D)).astype(np.float32)
    filt = rng.standard_normal((D, L)).astype(np.float32)
    y = kernel(x, filt)
    print("y", y.shape, y.dtype, float(np.abs(y).max()))


# revision 6
# speedup vs baseline: 3.9092x; 1.1853x over previous
"""
LongConvolution (causal FFT conv) Trainium2 Bass kernel — v3.

Problem: x (4, 8192, 1024) f32, filt (1024, 8192) f32.
  y[b, l, c] = sum_m x[b, m, c] * filt[c, l - m]   (causal, per-channel)
Reference computes this via zero-padded FFT of size N = 16384 = 128*128.

Strategy
--------
1. Packed-complex batches: z = x[2p] + i*x[2p+1].  The filter multiply is
   linear, so IFFT(FFT(z) * K) = y[2p] + i*y[2p+1] with NO Hermitian
   unpacking.  4 real convs become 2 complex pipelines: halves F2/I1
   matmul work and ALL twiddle/pointwise elementwise work.
2. f16 everywhere on-chip: matmuls at 1 cycle/row, DVE elementwise at
   2x rate.  PSUM stays f32.
3. All 18 elementwise ops on DVE.  (v2 offloaded 4 to GpSimd: SBUF port
   contention stretched concurrent DVE ops to GpSimd's duration — net
   loss.  ACT does only PSUM->SBUF converting copies.)
4. Stacked stationaries kill matmuls+LDWEIGHTS: F1 contracts the
   stacked [z_re; z_im] (K=128, one matmul per packed signal); I2 uses
   [gc|gs] / [-gs|gc] so Re and Im outputs come from one matmul pair,
   with two channels sharing a PSUM bank along the free dim.
5. 4-channel iterations: elementwise free-size 1024; per-channel PSUM
   tiles (1 bank) so all stages double-buffer in 8 banks.

Four-step FFT (k = 128*k2 + k1, n = 128*n1 + n2):
  A[n2,k1] = sum_n1 e^{-2pi i n1 k1/128} z[128 n1 + n2]     (F1, K=64x2)
  B = A * T1,  T1[n2,k1] = e^{-2pi i n2 k1/N}               (T1, DVE)
  R[k2,k1] = sum_n2 e^{-2pi i n2 k2/128} B[n2,k1]           (F2)
  P = R * K   (filter spectrum, host-precomputed, [k2,k1])  (PW)
  C[k1,n2] = sum_k2 e^{+2pi i n2 k2/128} P[k2,k1]           (I1)
  C' = C * conj(T1)  ([k1,n2] layout, T1 symmetric)         (T2)
  y[n1,n2] = sum_k1 e^{+2pi i n1 k1/128} C'[k1,n2], n1<64   (I2)
  y[2p] = Re, y[2p+1] = Im.

Sharding: d_model across the 8 cores (128 channels each).
"""

import os
import sys

import numpy as np

for p in ("/opt/trn_rl_repo",):
    if p not in sys.path:
        sys.path.insert(0, p)

os.environ.setdefault("MYCRO_LOCAL_CACHE", "1")

# ----------------------------------------------------------------------------
# configuration
# ----------------------------------------------------------------------------
B, L, D = 4, 8192, 1024
NFFT = 2 * L               # 16384 = 128 * 128
NC = 8                     # cores
CPC = D // NC              # channels per core = 128
CHG = 4                    # channels per iteration
NIT = CPC // CHG           # 32 iterations


def _consts():
    j = np.arange(128)
    ang128 = 2 * np.pi * np.outer(j, j) / 128
    angN = 2 * np.pi * np.outer(j, j) / NFFT
    return {
        "F_cos": np.cos(ang128), "F_sin": np.sin(ang128),
        "Tw_cos": np.cos(angN), "Tw_sin": np.sin(angN),
    }


def _build_program():
    import concourse.bacc as bacc
    import concourse.mybir as mybir
    from concourse import tile

    f32 = mybir.dt.float32
    f16 = mybir.dt.float16

    nc = bacc.Bacc(None, target_bir_lowering=False, debug=False)

    # --- DRAM I/O (all f16) ---
    # xw[it, (ri,n1), 2*chl+pk, n2] — stacked z_re/z_im chunks
    xw = nc.dram_tensor("xw", (NIT, 128, 2 * CHG, 128), f16, kind="ExternalInput")
    # kk[it, k2, chl, ri, k1]
    kk = nc.dram_tensor("kk", (NIT, 128, CHG, 2, 128), f16, kind="ExternalInput")
    f1m_d = nc.dram_tensor("f1m", (128, 256), f16, kind="ExternalInput")
    f2c_d = nc.dram_tensor("f2c", (128, 128), f16, kind="ExternalInput")
    f2s_d = nc.dram_tensor("f2s", (128, 128), f16, kind="ExternalInput")
    f2sn_d = nc.dram_tensor("f2sn", (128, 128), f16, kind="ExternalInput")
    fim_d = nc.dram_tensor("fim", (128, 512), f16, kind="ExternalInput")
    gcs_d = nc.dram_tensor("gcs", (128, 128), f16, kind="ExternalInput")
    gnc_d = nc.dram_tensor("gnc", (128, 128), f16, kind="ExternalInput")
    t1re_d = nc.dram_tensor("t1re", (128, 128), f16, kind="ExternalInput")
    t1im_d = nc.dram_tensor("t1im", (128, 128), f16, kind="ExternalInput")
    # yw[pair, cl, (sig,n1), pk, n2] — two channels share one PSUM bank
    yw = nc.dram_tensor(
        "yw", (CPC // 2, 2, 128, 2, 128), f16, kind="ExternalOutput"
    )

    with tile.TileContext(nc) as tc:
        with (
            tc.tile_pool(name="const", bufs=1) as constp,
            tc.tile_pool(name="m", bufs=2) as mp,
            tc.tile_pool(name="kf", bufs=2) as kp,
            tc.tile_pool(name="work", bufs=3) as wp,
            tc.tile_pool(name="scr", bufs=4) as up,
            tc.tile_pool(name="out", bufs=3) as op,
            tc.tile_pool(name="pa", bufs=2, space="PSUM") as pap,
            tc.tile_pool(name="pr", bufs=2, space="PSUM") as prp,
            tc.tile_pool(name="pc", bufs=2, space="PSUM") as pcp,
            tc.tile_pool(name="py", bufs=2, space="PSUM") as pyp,
        ):
            f1m = constp.tile([128, 256], f16)
            f2c = constp.tile([128, 128], f16)
            f2s = constp.tile([128, 128], f16)
            f2sn = constp.tile([128, 128], f16)
            fim = constp.tile([128, 512], f16)
            gcs = constp.tile([128, 128], f16)
            gnc = constp.tile([128, 128], f16)
            t1re = constp.tile([128, 128], f16)
            t1im = constp.tile([128, 128], f16)
            for t, d in (
                (f1m, f1m_d), (f2c, f2c_d), (f2s, f2s_d), (f2sn, f2sn_d),
                (fim, fim_d), (gcs, gcs_d), (gnc, gnc_d),
                (t1re, t1re_d), (t1im, t1im_d),
            ):
                nc.sync.dma_start(t[:], d[:])
            t1re_b = (
                t1re[:].rearrange("p (s n) -> p s n", s=1)
                .broadcast_to([128, 2 * CHG, 128])
            )
            t1im_b = (
                t1im[:].rearrange("p (s n) -> p s n", s=1)
                .broadcast_to([128, 2 * CHG, 128])
            )

            for it in range(NIT):
                m4 = mp.tile([128, 2 * CHG, 128], f16, tag="m")
                nc.sync.dma_start(m4[:], xw[it])
                kt = kp.tile([128, CHG, 2, 128], f16, tag="k")
                nc.sync.dma_start(kt[:], kk[it])

                # ---- F1 (stacked K=128) + copy: pa -> asrc ----
                asrc = wp.tile([128, 2, 2 * CHG, 128], f16, tag="asrc")
                for ch in range(CHG):
                    pa = pap.tile([128, 2, 2, 128], f32, tag="pa")
                    for pk in range(2):
                        nc.tensor.matmul(
                            pa[:, :, pk, :], m4[:, 2 * ch + pk, :], f1m[:],
                            start=True, stop=True,
                        )
                    nc.scalar.copy(
                        out=asrc[:, :, 2 * ch : 2 * ch + 2, :], in_=pa[:]
                    )

                # ---- T1 twiddle (DVE) ----
                a_re = asrc[:, 0, :, :]
                a_im = asrc[:, 1, :, :]
                b_t = wp.tile([128, 2, 2 * CHG, 128], f16, tag="b")
                u1 = up.tile([128, 2 * CHG, 128], f16, tag="u1")
                u2 = up.tile([128, 2 * CHG, 128], f16, tag="u2")
                u3 = up.tile([128, 2 * CHG, 128], f16, tag="u3")
                u4 = up.tile([128, 2 * CHG, 128], f16, tag="u4")
                nc.vector.tensor_mul(u1[:], a_re, t1re_b)
                nc.vector.tensor_mul(u2[:], a_im, t1im_b)
                nc.vector.tensor_mul(u3[:], a_re, t1im_b)
                nc.vector.tensor_mul(u4[:], a_im, t1re_b)
                nc.vector.tensor_sub(b_t[:, 0, :, :], u1[:], u2[:])
                nc.vector.tensor_add(b_t[:, 1, :, :], u3[:], u4[:])

                # ---- F2 + copy: pr -> rsrc ----
                rsrc = wp.tile([128, 2, 2 * CHG, 128], f16, tag="rsrc")
                for ch in range(CHG):
                    pr = prp.tile([128, 2, 2, 128], f32, tag="pr")
                    for pk in range(2):
                        j = 2 * ch + pk
                        b_re = b_t[:, 0, j, :]
                        b_im = b_t[:, 1, j, :]
                        nc.tensor.matmul(
                            pr[:, 0, pk, :], f2c[:], b_re, start=True, stop=False
                        )
                        nc.tensor.matmul(
                            pr[:, 0, pk, :], f2s[:], b_im, start=False, stop=True
                        )
                        nc.tensor.matmul(
                            pr[:, 1, pk, :], f2c[:], b_im, start=True, stop=False
                        )
                        nc.tensor.matmul(
                            pr[:, 1, pk, :], f2sn[:], b_re, start=False, stop=True
                        )
                    nc.scalar.copy(
                        out=rsrc[:, :, 2 * ch : 2 * ch + 2, :], in_=pr[:]
                    )

                # ---- pointwise filter multiply (DVE) ----
                r_re = rsrc[:, 0, :, :].rearrange("p (c k) n -> p c k n", c=CHG)
                r_im = rsrc[:, 1, :, :].rearrange("p (c k) n -> p c k n", c=CHG)
                kre_b = (
                    kt[:, :, 0, :].rearrange("p c (s n) -> p c s n", s=1)
                    .broadcast_to([128, CHG, 2, 128])
                )
                kim_b = (
                    kt[:, :, 1, :].rearrange("p c (s n) -> p c s n", s=1)
                    .broadcast_to([128, CHG, 2, 128])
                )
                p_t = wp.tile([128, 2, 2 * CHG, 128], f16, tag="p")
                p_re = p_t[:, 0, :, :].rearrange("p (c k) n -> p c k n", c=CHG)
                p_im = p_t[:, 1, :, :].rearrange("p (c k) n -> p c k n", c=CHG)
                v1 = up.tile([128, CHG, 2, 128], f16, tag="u1")
                v2 = up.tile([128, CHG, 2, 128], f16, tag="u2")
                v3 = up.tile([128, CHG, 2, 128], f16, tag="u3")
                v4 = up.tile([128, CHG, 2, 128], f16, tag="u4")
                nc.vector.tensor_mul(v1[:], r_re, kre_b)
                nc.vector.tensor_mul(v2[:], r_im, kim_b)
                nc.vector.tensor_sub(p_re, v1[:], v2[:])
                nc.vector.tensor_mul(v3[:], r_re, kim_b)
                nc.vector.tensor_mul(v4[:], r_im, kre_b)
                nc.vector.tensor_add(p_im, v3[:], v4[:])

                # ---- I1 + copy: pc -> csrc ----
                csrc = wp.tile([128, 2, 2 * CHG, 128], f16, tag="csrc")
                for ch in range(CHG):
                    pc = pcp.tile([128, 2, 2, 128], f32, tag="pc")
                    for pk in range(2):
                        j = 2 * ch + pk
                        nc.tensor.matmul(
                            pc[:, :, pk, :], p_t[:, 0, j, :], fim[:, 0:256],
                            start=True, stop=False,
                        )
                        nc.tensor.matmul(
                            pc[:, :, pk, :], p_t[:, 1, j, :], fim[:, 256:512],
                            start=False, stop=True,
                        )
                    nc.scalar.copy(
                        out=csrc[:, :, 2 * ch : 2 * ch + 2, :], in_=pc[:]
                    )

                # ---- T2 twiddle (DVE) ----
                c_re = csrc[:, 0, :, :]
                c_im = csrc[:, 1, :, :]
                cp_t = wp.tile([128, 2, 2 * CHG, 128], f16, tag="cp")
                w1 = up.tile([128, 2 * CHG, 128], f16, tag="u1")
                w2 = up.tile([128, 2 * CHG, 128], f16, tag="u2")
                w3 = up.tile([128, 2 * CHG, 128], f16, tag="u3")
                w4 = up.tile([128, 2 * CHG, 128], f16, tag="u4")
                nc.vector.tensor_mul(w1[:], c_re, t1re_b)
                nc.vector.tensor_mul(w2[:], c_im, t1im_b)
                nc.vector.tensor_mul(w3[:], c_re, t1im_b)
                nc.vector.tensor_mul(w4[:], c_im, t1re_b)
                nc.vector.tensor_add(cp_t[:, 0, :, :], w1[:], w2[:])
                nc.vector.tensor_sub(cp_t[:, 1, :, :], w4[:], w3[:])

                # ---- I2 (stacked [gc|gs]) + copy + DMA out ----
                for ch in range(CHG):
                    cl = ch % 2
                    if cl == 0:
                        py = pyp.tile([128, 2, 2, 128], f32, tag="py")
                    for pk in range(2):
                        j = 2 * ch + pk
                        nc.tensor.matmul(
                            py[:, cl, pk, :], gcs[:], cp_t[:, 0, j, :],
                            start=True, stop=False,
                        )
                        nc.tensor.matmul(
                            py[:, cl, pk, :], gnc[:], cp_t[:, 1, j, :],
                            start=False, stop=True,
                        )
                    if cl == 1:
                        pair = (CHG * it + ch) // 2
                        ysb = op.tile([128, 2, 2, 128], f16, tag="ysb")
                        nc.scalar.copy(out=ysb[:], in_=py[:])
                        nc.sync.dma_start(
                            yw[pair].rearrange("c p k n -> p c k n"), ysb[:]
                        )

    nc.compile()
    return nc


def _host_arrays():
    cst = _consts()
    F_cos, F_sin = cst["F_cos"], cst["F_sin"]
    Tw_cos, Tw_sin = cst["Tw_cos"], cst["Tw_sin"]
    f16 = np.float16
    cosF, sinF = F_cos[:64, :], F_sin[:64, :]
    arrs = {}
    # stacked F1 moving: rows 0:64 act on z_re, rows 64:128 on z_im
    arrs["f1m"] = np.block([[cosF, -sinF], [sinF, cosF]]).astype(f16)
    arrs["f2c"] = F_cos.astype(f16)
    arrs["f2s"] = F_sin.astype(f16)
    arrs["f2sn"] = (-F_sin).astype(f16)
    arrs["fim"] = np.concatenate(
        [F_cos, F_sin, -F_sin, F_cos], axis=1
    ).astype(f16)
    # stacked I2 stationaries: out partitions 0:64 = Re (y even batch),
    # 64:128 = Im (y odd batch)
    arrs["gcs"] = np.concatenate(
        [F_cos[:, :64], F_sin[:, :64]], axis=1
    ).astype(f16)
    arrs["gnc"] = np.concatenate(
        [-F_sin[:, :64], F_cos[:, :64]], axis=1
    ).astype(f16)
    arrs["t1re"] = Tw_cos.astype(f16)
    arrs["t1im"] = (-Tw_sin).astype(f16)
    return arrs


def _prep_inputs(x, filt):
    """Full inputs -> list of per-core input maps."""
    consts = _host_arrays()

    kpad = np.zeros((D, NFFT), np.float64)
    kpad[:, :L] = filt
    Kf = (np.fft.fft(kpad, axis=1) / NFFT).reshape(D, 128, 128)  # [c, k2, k1]

    # x -> (D, 2pk, 2ri, 64 n1, 128 n2)
    xq = np.ascontiguousarray(x.transpose(2, 0, 1)).reshape(D, 2, 2, 64, 128)

    in_maps = []
    for ci in range(NC):
        sl = slice(ci * CPC, (ci + 1) * CPC)
        m = dict(consts)
        xc = xq[sl].reshape(NIT, CHG, 2, 2, 64, 128)
        # -> (it, (ri,n1), (chl,pk), n2)
        m["xw"] = np.ascontiguousarray(
            xc.transpose(0, 3, 4, 1, 2, 5).reshape(NIT, 128, 2 * CHG, 128)
        ).astype(np.float16)
        kc = Kf[sl]
        kri = np.stack([kc.real, kc.imag], axis=1)  # (CPC, 2ri, 128k2, 128k1)
        m["kk"] = np.ascontiguousarray(
            kri.reshape(NIT, CHG, 2, 128, 128).transpose(0, 3, 1, 2, 4)
        ).astype(np.float16)
        in_maps.append(m)
    return in_maps


def _post_outputs(res):
    y = np.empty((B, L, D), np.float32)
    for ci in range(NC):
        sl = slice(ci * CPC, (ci + 1) * CPC)
        # (pair, cl, (sig,n1), pk, n2); c = 2*pair+cl, b = 2*pk+sig,
        # l = 128*n1+n2
        r = res.results[ci]["yw"].astype(np.float32)
        r = r.reshape(CPC // 2, 2, 2, 64, 2, 128)
        r = r.transpose(4, 2, 3, 5, 0, 1).reshape(B, L, CPC)
        y[:, :, sl] = r
    return y


def kernel(x: np.ndarray, filt: np.ndarray) -> np.ndarray:
    from concourse.bass_utils import run_bass_kernel_spmd

    assert x.shape == (B, L, D) and filt.shape == (D, L)
    x = np.ascontiguousarray(x, dtype=np.float32)
    filt = np.ascontiguousarray(filt, dtype=np.float32)

    in_maps = _prep_inputs(x, filt)
    nc = _build_program()
    res = run_bass_kernel_spmd(nc, in_maps, core_ids=list(range(NC)))
    return _post_outputs(res)


def run_profiled(inputs):
    """Build + run with NTFF tracing; returns BassKernelResults (test-only)."""
    from concourse.bass_utils import run_bass_kernel_spmd

    x = np.ascontiguousarray(inputs["x"], dtype=np.float32)
    filt = np.ascontiguousarray(inputs["filt"], dtype=np.float32)
    in_maps = _prep_inputs(x, filt)
    nc = _build_program()
    return run_bass_kernel_spmd(
        nc, in_maps, core_ids=list(range(NC)), trace=True
    )


if __name__ == "__main__":
    rng = np.random.default_rng(0)
    x = rng.standard_normal((B, L, D)).astype(np.float32)
    filt = rng.standard_normal((D, L)).astype(np.float32)
    y = kernel(x, filt)
    print("y", y.shape, y.dtype, float(np.abs(y).max()))


# revision 8
# speedup vs baseline: 6.6095x; 1.6907x over previous
"""
LongConvolution (causal FFT conv) Trainium2 Bass kernel — v3.

Problem: x (4, 8192, 1024) f32, filt (1024, 8192) f32.
  y[b, l, c] = sum_m x[b, m, c] * filt[c, l - m]   (causal, per-channel)
Reference computes this via zero-padded FFT of size N = 16384 = 128*128.

Strategy
--------
1. Packed-complex batches: z = x[2p] + i*x[2p+1].  The filter multiply is
   linear, so IFFT(FFT(z) * K) = y[2p] + i*y[2p+1] with NO Hermitian
   unpacking.  4 real convs become 2 complex pipelines: halves F2/I1
   matmul work and ALL twiddle/pointwise elementwise work.
2. f16 everywhere on-chip: matmuls at 1 cycle/row, DVE elementwise at
   2x rate.  PSUM stays f32.
3. All 18 elementwise ops on DVE.  (v2 offloaded 4 to GpSimd: SBUF port
   contention stretched concurrent DVE ops to GpSimd's duration — net
   loss.  ACT does only PSUM->SBUF converting copies.)
4. Stacked stationaries kill matmuls+LDWEIGHTS: F1 contracts the
   stacked [z_re; z_im] (K=128, one matmul per packed signal); I2 uses
   [gc|gs] / [-gs|gc] so Re and Im outputs come from one matmul pair,
   with two channels sharing a PSUM bank along the free dim.
5. 4-channel iterations: elementwise free-size 1024; per-channel PSUM
   tiles (1 bank) so all stages double-buffer in 8 banks.

Four-step FFT (k = 128*k2 + k1, n = 128*n1 + n2):
  A[n2,k1] = sum_n1 e^{-2pi i n1 k1/128} z[128 n1 + n2]     (F1, K=64x2)
  B = A * T1,  T1[n2,k1] = e^{-2pi i n2 k1/N}               (T1, DVE)
  R[k2,k1] = sum_n2 e^{-2pi i n2 k2/128} B[n2,k1]           (F2)
  P = R * K   (filter spectrum, host-precomputed, [k2,k1])  (PW)
  C[k1,n2] = sum_k2 e^{+2pi i n2 k2/128} P[k2,k1]           (I1)
  C' = C * conj(T1)  ([k1,n2] layout, T1 symmetric)         (T2)
  y[n1,n2] = sum_k1 e^{+2pi i n1 k1/128} C'[k1,n2], n1<64   (I2)
  y[2p] = Re, y[2p+1] = Im.

Sharding: d_model across the 8 cores (128 channels each).
"""

import os
import sys

import numpy as np

for p in ("/opt/trn_rl_repo",):
    if p not in sys.path:
        sys.path.insert(0, p)

os.environ.setdefault("MYCRO_LOCAL_CACHE", "1")

# ----------------------------------------------------------------------------
# configuration
# ----------------------------------------------------------------------------
B, L, D = 4, 8192, 1024
NFFT = 2 * L               # 16384 = 128 * 128
NC = 8                     # cores
CPC = D // NC              # channels per core = 128
CHG = 4                    # channels per iteration
NIT = CPC // CHG           # 32 iterations


def _consts():
    j = np.arange(128)
    ang128 = 2 * np.pi * np.outer(j, j) / 128
    angN = 2 * np.pi * np.outer(j, j) / NFFT
    return {
        "F_cos": np.cos(ang128), "F_sin": np.sin(ang128),
        "Tw_cos": np.cos(angN), "Tw_sin": np.sin(angN),
    }


def _build_program():
    import concourse.bacc as bacc
    import concourse.mybir as mybir
    from concourse import tile

    f32 = mybir.dt.float32
    f16 = mybir.dt.float16

    nc = bacc.Bacc(None, target_bir_lowering=False, debug=False)

    # --- DRAM I/O (all f16) ---
    # xw[it, (ri,n1), 2*chl+pk, n2] — stacked z_re/z_im chunks
    xw = nc.dram_tensor("xw", (NIT, 128, 2 * CHG, 128), f16, kind="ExternalInput")
    # kk[it, k2, chl, ri, k1]
    kk = nc.dram_tensor("kk", (NIT, 128, CHG, 2, 128), f16, kind="ExternalInput")
    f1m_d = nc.dram_tensor("f1m", (128, 256), f16, kind="ExternalInput")
    f2c_d = nc.dram_tensor("f2c", (128, 128), f16, kind="ExternalInput")
    f2s_d = nc.dram_tensor("f2s", (128, 128), f16, kind="ExternalInput")
    f2sn_d = nc.dram_tensor("f2sn", (128, 128), f16, kind="ExternalInput")
    fim_d = nc.dram_tensor("fim", (128, 512), f16, kind="ExternalInput")
    gcs_d = nc.dram_tensor("gcs", (128, 128), f16, kind="ExternalInput")
    gnc_d = nc.dram_tensor("gnc", (128, 128), f16, kind="ExternalInput")
    t1re_d = nc.dram_tensor("t1re", (128, 128), f16, kind="ExternalInput")
    t1im_d = nc.dram_tensor("t1im", (128, 128), f16, kind="ExternalInput")
    # yw[pair, cl, (sig,n1), pk, n2] — two channels share one PSUM bank
    yw = nc.dram_tensor(
        "yw", (CPC // 2, 2, 128, 2, 128), f16, kind="ExternalOutput"
    )

    with tile.TileContext(nc) as tc:
        with (
            tc.tile_pool(name="const", bufs=1) as constp,
            tc.tile_pool(name="m", bufs=2) as mp,
            tc.tile_pool(name="kf", bufs=2) as kp,
            tc.tile_pool(name="work", bufs=3) as wp,
            tc.tile_pool(name="scr", bufs=4) as up,
            tc.tile_pool(name="out", bufs=3) as op,
            tc.tile_pool(name="pa", bufs=2, space="PSUM") as pap,
            tc.tile_pool(name="pr", bufs=2, space="PSUM") as prp,
            tc.tile_pool(name="pc", bufs=2, space="PSUM") as pcp,
            tc.tile_pool(name="py", bufs=2, space="PSUM") as pyp,
        ):
            f1m = constp.tile([128, 256], f16)
            f2c = constp.tile([128, 128], f16)
            f2s = constp.tile([128, 128], f16)
            f2sn = constp.tile([128, 128], f16)
            fim = constp.tile([128, 512], f16)
            gcs = constp.tile([128, 128], f16)
            gnc = constp.tile([128, 128], f16)
            t1re = constp.tile([128, 128], f16)
            t1im = constp.tile([128, 128], f16)
            for t, d in (
                (f1m, f1m_d), (f2c, f2c_d), (f2s, f2s_d), (f2sn, f2sn_d),
                (fim, fim_d), (gcs, gcs_d), (gnc, gnc_d),
                (t1re, t1re_d), (t1im, t1im_d),
            ):
                nc.sync.dma_start(t[:], d[:])
            t1re_b = (
                t1re[:].rearrange("p (s n) -> p s n", s=1)
                .broadcast_to([128, 2 * CHG, 128])
            )
            t1im_b = (
                t1im[:].rearrange("p (s n) -> p s n", s=1)
                .broadcast_to([128, 2 * CHG, 128])
            )

            # Per-iteration stage emitters.  Stages of iteration pairs are
            # emitted interleaved (2-wide software pipelining) so every
            # engine's in-order stream has independent work between
            # dependent stages of one iteration.
            st = {}  # it -> dict of live tiles

            def e_dma(it):
                s = st[it] = {}
                s["m4"] = mp.tile([128, 2 * CHG, 128], f16, tag="m", name="m4")
                nc.sync.dma_start(s["m4"][:], xw[it])
                s["kt"] = kp.tile([128, CHG, 2, 128], f16, tag="k", name="kt")
                nc.sync.dma_start(s["kt"][:], kk[it])

            def e_f1(it):
                s = st[it]
                s["asrc"] = wp.tile([128, 2, 2 * CHG, 128], f16, tag="asrc", name="asrc")
                for ch in range(CHG):
                    pa = pap.tile([128, 2, 2, 128], f32, tag="pa")
                    for pk in range(2):
                        nc.tensor.matmul(
                            pa[:, :, pk, :], s["m4"][:, 2 * ch + pk, :],
                            f1m[:], start=True, stop=True,
                        )
                    nc.scalar.copy(
                        out=s["asrc"][:, :, 2 * ch : 2 * ch + 2, :], in_=pa[:]
                    )

            def e_t1(it):
                s = st[it]
                a_re = s["asrc"][:, 0, :, :]
                a_im = s["asrc"][:, 1, :, :]
                b_t = s["b"] = wp.tile([128, 2, 2 * CHG, 128], f16, tag="b", name="b_t")
                u1 = up.tile([128, 2 * CHG, 128], f16, tag="u1")
                u2 = up.tile([128, 2 * CHG, 128], f16, tag="u2")
                u3 = up.tile([128, 2 * CHG, 128], f16, tag="u3")
                u4 = up.tile([128, 2 * CHG, 128], f16, tag="u4")
                nc.vector.tensor_mul(u1[:], a_re, t1re_b)
                nc.vector.tensor_mul(u2[:], a_im, t1im_b)
                nc.vector.tensor_mul(u3[:], a_re, t1im_b)
                nc.vector.tensor_mul(u4[:], a_im, t1re_b)
                nc.vector.tensor_sub(b_t[:, 0, :, :], u1[:], u2[:])
                nc.vector.tensor_add(b_t[:, 1, :, :], u3[:], u4[:])

            def e_f2(it):
                s = st[it]
                b_t = s["b"]
                s["rsrc"] = wp.tile([128, 2, 2 * CHG, 128], f16, tag="rsrc", name="rsrc")
                for ch in range(CHG):
                    pr = prp.tile([128, 2, 2, 128], f32, tag="pr")
                    for pk in range(2):
                        j = 2 * ch + pk
                        b_re = b_t[:, 0, j, :]
                        b_im = b_t[:, 1, j, :]
                        nc.tensor.matmul(
                            pr[:, 0, pk, :], f2c[:], b_re,
                            start=True, stop=False,
                        )
                        nc.tensor.matmul(
                            pr[:, 0, pk, :], f2s[:], b_im,
                            start=False, stop=True,
                        )
                        nc.tensor.matmul(
                            pr[:, 1, pk, :], f2c[:], b_im,
                            start=True, stop=False,
                        )
                        nc.tensor.matmul(
                            pr[:, 1, pk, :], f2sn[:], b_re,
                            start=False, stop=True,
                        )
                    nc.scalar.copy(
                        out=s["rsrc"][:, :, 2 * ch : 2 * ch + 2, :], in_=pr[:]
                    )

            def e_pw(it):
                s = st[it]
                rsrc, kt = s["rsrc"], s["kt"]
                r_re = rsrc[:, 0, :, :].rearrange("p (c k) n -> p c k n", c=CHG)
                r_im = rsrc[:, 1, :, :].rearrange("p (c k) n -> p c k n", c=CHG)
                kre_b = (
                    kt[:, :, 0, :].rearrange("p c (s n) -> p c s n", s=1)
                    .broadcast_to([128, CHG, 2, 128])
                )
                kim_b = (
                    kt[:, :, 1, :].rearrange("p c (s n) -> p c s n", s=1)
                    .broadcast_to([128, CHG, 2, 128])
                )
                p_t = s["p"] = wp.tile([128, 2, 2 * CHG, 128], f16, tag="p", name="p_t")
                p_re = p_t[:, 0, :, :].rearrange("p (c k) n -> p c k n", c=CHG)
                p_im = p_t[:, 1, :, :].rearrange("p (c k) n -> p c k n", c=CHG)
                v1 = up.tile([128, CHG, 2, 128], f16, tag="u1")
                v2 = up.tile([128, CHG, 2, 128], f16, tag="u2")
                v3 = up.tile([128, CHG, 2, 128], f16, tag="u3")
                v4 = up.tile([128, CHG, 2, 128], f16, tag="u4")
                nc.vector.tensor_mul(v1[:], r_re, kre_b)
                nc.vector.tensor_mul(v2[:], r_im, kim_b)
                nc.vector.tensor_sub(p_re, v1[:], v2[:])
                nc.vector.tensor_mul(v3[:], r_re, kim_b)
                nc.vector.tensor_mul(v4[:], r_im, kre_b)
                nc.vector.tensor_add(p_im, v3[:], v4[:])

            def e_i1(it):
                s = st[it]
                p_t = s["p"]
                s["csrc"] = wp.tile([128, 2, 2 * CHG, 128], f16, tag="csrc", name="csrc")
                for ch in range(CHG):
                    pc = pcp.tile([128, 2, 2, 128], f32, tag="pc")
                    for pk in range(2):
                        j = 2 * ch + pk
                        nc.tensor.matmul(
                            pc[:, :, pk, :], p_t[:, 0, j, :], fim[:, 0:256],
                            start=True, stop=False,
                        )
                        nc.tensor.matmul(
                            pc[:, :, pk, :], p_t[:, 1, j, :], fim[:, 256:512],
                            start=False, stop=True,
                        )
                    nc.scalar.copy(
                        out=s["csrc"][:, :, 2 * ch : 2 * ch + 2, :], in_=pc[:]
                    )

            def e_t2(it):
                s = st[it]
                c_re = s["csrc"][:, 0, :, :]
                c_im = s["csrc"][:, 1, :, :]
                cp_t = s["cp"] = wp.tile([128, 2, 2 * CHG, 128], f16, tag="cp", name="cp_t")
                w1 = up.tile([128, 2 * CHG, 128], f16, tag="u1")
                w2 = up.tile([128, 2 * CHG, 128], f16, tag="u2")
                w3 = up.tile([128, 2 * CHG, 128], f16, tag="u3")
                w4 = up.tile([128, 2 * CHG, 128], f16, tag="u4")
                nc.vector.tensor_mul(w1[:], c_re, t1re_b)
                nc.vector.tensor_mul(w2[:], c_im, t1im_b)
                nc.vector.tensor_mul(w3[:], c_re, t1im_b)
                nc.vector.tensor_mul(w4[:], c_im, t1re_b)
                nc.vector.tensor_add(cp_t[:, 0, :, :], w1[:], w2[:])
                nc.vector.tensor_sub(cp_t[:, 1, :, :], w4[:], w3[:])

            def e_i2(it):
                s = st[it]
                cp_t = s["cp"]
                py = None
                for ch in range(CHG):
                    cl = ch % 2
                    if cl == 0:
                        py = pyp.tile([128, 2, 2, 128], f32, tag="py")
                    for pk in range(2):
                        j = 2 * ch + pk
                        nc.tensor.matmul(
                            py[:, cl, pk, :], gcs[:], cp_t[:, 0, j, :],
                            start=True, stop=False,
                        )
                        nc.tensor.matmul(
                            py[:, cl, pk, :], gnc[:], cp_t[:, 1, j, :],
                            start=False, stop=True,
                        )
                    if cl == 1:
                        pair = (CHG * it + ch) // 2
                        ysb = op.tile([128, 2, 2, 128], f16, tag="ysb")
                        nc.scalar.copy(out=ysb[:], in_=py[:])
                        nc.sync.dma_start(
                            yw[pair].rearrange("c p k n -> p c k n"), ysb[:]
                        )
                del st[it]

            for pi in range(NIT // 2):
                e, o = 2 * pi, 2 * pi + 1
                e_dma(e)
                e_dma(o)
                e_f1(e)
                e_f1(o)
                e_t1(e)
                e_f2(e)
                e_t1(o)
                e_f2(o)
                e_pw(e)
                e_i1(e)
                e_pw(o)
                e_i1(o)
                e_t2(e)
                e_i2(e)
                e_t2(o)
                e_i2(o)

    nc.compile()
    return nc


def _host_arrays():
    cst = _consts()
    F_cos, F_sin = cst["F_cos"], cst["F_sin"]
    Tw_cos, Tw_sin = cst["Tw_cos"], cst["Tw_sin"]
    f16 = np.float16
    cosF, sinF = F_cos[:64, :], F_sin[:64, :]
    arrs = {}
    # stacked F1 moving: rows 0:64 act on z_re, rows 64:128 on z_im
    arrs["f1m"] = np.block([[cosF, -sinF], [sinF, cosF]]).astype(f16)
    arrs["f2c"] = F_cos.astype(f16)
    arrs["f2s"] = F_sin.astype(f16)
    arrs["f2sn"] = (-F_sin).astype(f16)
    arrs["fim"] = np.concatenate(
        [F_cos, F_sin, -F_sin, F_cos], axis=1
    ).astype(f16)
    # stacked I2 stationaries: out partitions 0:64 = Re (y even batch),
    # 64:128 = Im (y odd batch)
    arrs["gcs"] = np.concatenate(
        [F_cos[:, :64], F_sin[:, :64]], axis=1
    ).astype(f16)
    arrs["gnc"] = np.concatenate(
        [-F_sin[:, :64], F_cos[:, :64]], axis=1
    ).astype(f16)
    arrs["t1re"] = Tw_cos.astype(f16)
    arrs["t1im"] = (-Tw_sin).astype(f16)
    return arrs


def _prep_inputs(x, filt):
    """Full inputs -> list of per-core input maps."""
    consts = _host_arrays()

    kpad = np.zeros((D, NFFT), np.float64)
    kpad[:, :L] = filt
    Kf = (np.fft.fft(kpad, axis=1) / NFFT).reshape(D, 128, 128)  # [c, k2, k1]

    # x -> (D, 2pk, 2ri, 64 n1, 128 n2)
    xq = np.ascontiguousarray(x.transpose(2, 0, 1)).reshape(D, 2, 2, 64, 128)

    in_maps = []
    for ci in range(NC):
        sl = slice(ci * CPC, (ci + 1) * CPC)
        m = dict(consts)
        xc = xq[sl].reshape(NIT, CHG, 2, 2, 64, 128)
        # -> (it, (ri,n1), (chl,pk), n2)
        m["xw"] = np.ascontiguousarray(
            xc.transpose(0, 3, 4, 1, 2, 5).reshape(NIT, 128, 2 * CHG, 128)
        ).astype(np.float16)
        kc = Kf[sl]
        kri = np.stack([kc.real, kc.imag], axis=1)  # (CPC, 2ri, 128k2, 128k1)
        m["kk"] = np.ascontiguousarray(
            kri.reshape(NIT, CHG, 2, 128, 128).transpose(0, 3, 1, 2, 4)
        ).astype(np.float16)
        in_maps.append(m)
    return in_maps


def _post_outputs(res):
    y = np.empty((B, L, D), np.float32)
    for ci in range(NC):
        sl = slice(ci * CPC, (ci + 1) * CPC)
        # (pair, cl, (sig,n1), pk, n2); c = 2*pair+cl, b = 2*pk+sig,
        # l = 128*n1+n2
        r = res.results[ci]["yw"].astype(np.float32)
        r = r.reshape(CPC // 2, 2, 2, 64, 2, 128)
        r = r.transpose(4, 2, 3, 5, 0, 1).reshape(B, L, CPC)
        y[:, :, sl] = r
    return y


def kernel(x: np.ndarray, filt: np.ndarray) -> np.ndarray:
    from concourse.bass_utils import run_bass_kernel_spmd

    assert x.shape == (B, L, D) and filt.shape == (D, L)
    x = np.ascontiguousarray(x, dtype=np.float32)
    filt = np.ascontiguousarray(filt, dtype=np.float32)

    in_maps = _prep_inputs(x, filt)
    nc = _build_program()
    res = run_bass_kernel_spmd(nc, in_maps, core_ids=list(range(NC)))
    return _post_outputs(res)


def run_profiled(inputs):
    """Build + run with NTFF tracing; returns BassKernelResults (test-only)."""
    from concourse.bass_utils import run_bass_kernel_spmd

    x = np.ascontiguousarray(inputs["x"], dtype=np.float32)
    filt = np.ascontiguousarray(inputs["filt"], dtype=np.float32)
    in_maps = _prep_inputs(x, filt)
    nc = _build_program()
    return run_bass_kernel_spmd(
        nc, in_maps, core_ids=list(range(NC)), trace=True
    )


if __name__ == "__main__":
    rng = np.random.default_rng(0)
    x = rng.standard_normal((B, L, D)).astype(np.float32)
    filt = rng.standard_normal((D, L)).astype(np.float32)
    y = kernel(x, filt)
    print("y", y.shape, y.dtype, float(np.abs(y).max()))


# revision 9
# speedup vs baseline: 7.0512x; 1.0668x over previous
"""
LongConvolution (causal FFT conv) Trainium2 Bass kernel — v3.

Problem: x (4, 8192, 1024) f32, filt (1024, 8192) f32.
  y[b, l, c] = sum_m x[b, m, c] * filt[c, l - m]   (causal, per-channel)
Reference computes this via zero-padded FFT of size N = 16384 = 128*128.

Strategy
--------
1. Packed-complex batches: z = x[2p] + i*x[2p+1].  The filter multiply is
   linear, so IFFT(FFT(z) * K) = y[2p] + i*y[2p+1] with NO Hermitian
   unpacking.  4 real convs become 2 complex pipelines: halves F2/I1
   matmul work and ALL twiddle/pointwise elementwise work.
2. f16 everywhere on-chip: matmuls at 1 cycle/row, DVE elementwise at
   2x rate.  PSUM stays f32.
3. All 18 elementwise ops on DVE.  (v2 offloaded 4 to GpSimd: SBUF port
   contention stretched concurrent DVE ops to GpSimd's duration — net
   loss.  ACT does only PSUM->SBUF converting copies.)
4. Stacked stationaries kill matmuls+LDWEIGHTS: F1 contracts the
   stacked [z_re; z_im] (K=128, one matmul per packed signal); I2 uses
   [gc|gs] / [-gs|gc] so Re and Im outputs come from one matmul pair,
   with two channels sharing a PSUM bank along the free dim.
5. 4-channel iterations: elementwise free-size 1024; per-channel PSUM
   tiles (1 bank) so all stages double-buffer in 8 banks.

Four-step FFT (k = 128*k2 + k1, n = 128*n1 + n2):
  A[n2,k1] = sum_n1 e^{-2pi i n1 k1/128} z[128 n1 + n2]     (F1, K=64x2)
  B = A * T1,  T1[n2,k1] = e^{-2pi i n2 k1/N}               (T1, DVE)
  R[k2,k1] = sum_n2 e^{-2pi i n2 k2/128} B[n2,k1]           (F2)
  P = R * K   (filter spectrum, host-precomputed, [k2,k1])  (PW)
  C[k1,n2] = sum_k2 e^{+2pi i n2 k2/128} P[k2,k1]           (I1)
  C' = C * conj(T1)  ([k1,n2] layout, T1 symmetric)         (T2)
  y[n1,n2] = sum_k1 e^{+2pi i n1 k1/128} C'[k1,n2], n1<64   (I2)
  y[2p] = Re, y[2p+1] = Im.

Sharding: d_model across the 8 cores (128 channels each).
"""

import os
import sys

import numpy as np

for p in ("/opt/trn_rl_repo",):
    if p not in sys.path:
        sys.path.insert(0, p)

os.environ.setdefault("MYCRO_LOCAL_CACHE", "1")

# ----------------------------------------------------------------------------
# configuration
# ----------------------------------------------------------------------------
B, L, D = 4, 8192, 1024
NFFT = 2 * L               # 16384 = 128 * 128
NC = 8                     # cores
CPC = D // NC              # channels per core = 128
CHG = 8                    # channels per iteration
NIT = CPC // CHG           # 32 iterations


def _consts():
    j = np.arange(128)
    ang128 = 2 * np.pi * np.outer(j, j) / 128
    angN = 2 * np.pi * np.outer(j, j) / NFFT
    return {
        "F_cos": np.cos(ang128), "F_sin": np.sin(ang128),
        "Tw_cos": np.cos(angN), "Tw_sin": np.sin(angN),
    }


def _build_program():
    import concourse.bacc as bacc
    import concourse.mybir as mybir
    from concourse import tile

    f32 = mybir.dt.float32
    f16 = mybir.dt.float16

    nc = bacc.Bacc(None, target_bir_lowering=False, debug=False)

    # --- DRAM I/O (all f16) ---
    # xw[it, (ri,n1), 2*chl+pk, n2] — stacked z_re/z_im chunks
    xw = nc.dram_tensor("xw", (NIT, 128, 2 * CHG, 128), f16, kind="ExternalInput")
    # kk[it, k2, chl, ri, k1]
    kk = nc.dram_tensor("kk", (NIT, 128, CHG, 2, 128), f16, kind="ExternalInput")
    f1m_d = nc.dram_tensor("f1m", (128, 256), f16, kind="ExternalInput")
    f2c_d = nc.dram_tensor("f2c", (128, 128), f16, kind="ExternalInput")
    f2s_d = nc.dram_tensor("f2s", (128, 128), f16, kind="ExternalInput")
    f2sn_d = nc.dram_tensor("f2sn", (128, 128), f16, kind="ExternalInput")
    fim_d = nc.dram_tensor("fim", (128, 512), f16, kind="ExternalInput")
    gcs_d = nc.dram_tensor("gcs", (128, 128), f16, kind="ExternalInput")
    gnc_d = nc.dram_tensor("gnc", (128, 128), f16, kind="ExternalInput")
    t1re_d = nc.dram_tensor("t1re", (128, 128), f16, kind="ExternalInput")
    t1im_d = nc.dram_tensor("t1im", (128, 128), f16, kind="ExternalInput")
    # yw[pair, cl, (sig,n1), pk, n2] — two channels share one PSUM bank
    yw = nc.dram_tensor(
        "yw", (CPC // 2, 2, 128, 2, 128), f16, kind="ExternalOutput"
    )

    with tile.TileContext(nc) as tc:
        with (
            tc.tile_pool(name="const", bufs=1) as constp,
            tc.tile_pool(name="m", bufs=2) as mp,
            tc.tile_pool(name="kf", bufs=2) as kp,
            tc.tile_pool(name="work", bufs=2) as wp,
            tc.tile_pool(name="scr", bufs=4) as up,
            tc.tile_pool(name="out", bufs=3) as op,
            tc.tile_pool(name="pa", bufs=2, space="PSUM") as pap,
            tc.tile_pool(name="pr", bufs=2, space="PSUM") as prp,
            tc.tile_pool(name="pc", bufs=2, space="PSUM") as pcp,
            tc.tile_pool(name="py", bufs=2, space="PSUM") as pyp,
        ):
            f1m = constp.tile([128, 256], f16)
            f2c = constp.tile([128, 128], f16)
            f2s = constp.tile([128, 128], f16)
            f2sn = constp.tile([128, 128], f16)
            fim = constp.tile([128, 512], f16)
            gcs = constp.tile([128, 128], f16)
            gnc = constp.tile([128, 128], f16)
            t1re = constp.tile([128, 128], f16)
            t1im = constp.tile([128, 128], f16)
            for t, d in (
                (f1m, f1m_d), (f2c, f2c_d), (f2s, f2s_d), (f2sn, f2sn_d),
                (fim, fim_d), (gcs, gcs_d), (gnc, gnc_d),
                (t1re, t1re_d), (t1im, t1im_d),
            ):
                nc.sync.dma_start(t[:], d[:])
            t1re_b = (
                t1re[:].rearrange("p (s n) -> p s n", s=1)
                .broadcast_to([128, 2 * CHG, 128])
            )
            t1im_b = (
                t1im[:].rearrange("p (s n) -> p s n", s=1)
                .broadcast_to([128, 2 * CHG, 128])
            )

            # Per-iteration stage emitters.  Stages of iteration pairs are
            # emitted interleaved (2-wide software pipelining) so every
            # engine's in-order stream has independent work between
            # dependent stages of one iteration.
            st = {}  # it -> dict of live tiles

            def e_dma(it):
                s = st[it] = {}
                s["m4"] = mp.tile([128, 2 * CHG, 128], f16, tag="m", name="m4")
                nc.sync.dma_start(s["m4"][:], xw[it])
                s["kt"] = kp.tile([128, CHG, 2, 128], f16, tag="k", name="kt")
                nc.sync.dma_start(s["kt"][:], kk[it])

            def e_f1(it):
                s = st[it]
                s["asrc"] = wp.tile([128, 2, 2 * CHG, 128], f16, tag="asrc", name="asrc")
                for ch in range(CHG):
                    pa = pap.tile([128, 2, 2, 128], f32, tag="pa")
                    for pk in range(2):
                        nc.tensor.matmul(
                            pa[:, :, pk, :], s["m4"][:, 2 * ch + pk, :],
                            f1m[:], start=True, stop=True,
                        )
                    nc.scalar.copy(
                        out=s["asrc"][:, :, 2 * ch : 2 * ch + 2, :], in_=pa[:]
                    )

            def e_t1(it):
                s = st[it]
                a_re = s["asrc"][:, 0, :, :]
                a_im = s["asrc"][:, 1, :, :]
                b_t = s["b"] = wp.tile([128, 2, 2 * CHG, 128], f16, tag="b", name="b_t")
                u1 = up.tile([128, 2 * CHG, 128], f16, tag="u1")
                u2 = up.tile([128, 2 * CHG, 128], f16, tag="u2")
                u3 = up.tile([128, 2 * CHG, 128], f16, tag="u3")
                u4 = up.tile([128, 2 * CHG, 128], f16, tag="u4")
                nc.vector.tensor_mul(u1[:], a_re, t1re_b)
                nc.vector.tensor_mul(u2[:], a_im, t1im_b)
                nc.vector.tensor_mul(u3[:], a_re, t1im_b)
                nc.vector.tensor_mul(u4[:], a_im, t1re_b)
                nc.vector.tensor_sub(b_t[:, 0, :, :], u1[:], u2[:])
                nc.vector.tensor_add(b_t[:, 1, :, :], u3[:], u4[:])

            def e_f2(it):
                s = st[it]
                b_t = s["b"]
                s["rsrc"] = wp.tile([128, 2, 2 * CHG, 128], f16, tag="rsrc", name="rsrc")
                for ch in range(CHG):
                    pr = prp.tile([128, 2, 2, 128], f32, tag="pr")
                    for pk in range(2):
                        j = 2 * ch + pk
                        b_re = b_t[:, 0, j, :]
                        b_im = b_t[:, 1, j, :]
                        nc.tensor.matmul(
                            pr[:, 0, pk, :], f2c[:], b_re,
                            start=True, stop=False,
                        )
                        nc.tensor.matmul(
                            pr[:, 0, pk, :], f2s[:], b_im,
                            start=False, stop=True,
                        )
                        nc.tensor.matmul(
                            pr[:, 1, pk, :], f2c[:], b_im,
                            start=True, stop=False,
                        )
                        nc.tensor.matmul(
                            pr[:, 1, pk, :], f2sn[:], b_re,
                            start=False, stop=True,
                        )
                    nc.scalar.copy(
                        out=s["rsrc"][:, :, 2 * ch : 2 * ch + 2, :], in_=pr[:]
                    )

            def e_pw(it):
                s = st[it]
                rsrc, kt = s["rsrc"], s["kt"]
                r_re = rsrc[:, 0, :, :].rearrange("p (c k) n -> p c k n", c=CHG)
                r_im = rsrc[:, 1, :, :].rearrange("p (c k) n -> p c k n", c=CHG)
                kre_b = (
                    kt[:, :, 0, :].rearrange("p c (s n) -> p c s n", s=1)
                    .broadcast_to([128, CHG, 2, 128])
                )
                kim_b = (
                    kt[:, :, 1, :].rearrange("p c (s n) -> p c s n", s=1)
                    .broadcast_to([128, CHG, 2, 128])
                )
                p_t = s["p"] = wp.tile([128, 2, 2 * CHG, 128], f16, tag="p", name="p_t")
                p_re = p_t[:, 0, :, :].rearrange("p (c k) n -> p c k n", c=CHG)
                p_im = p_t[:, 1, :, :].rearrange("p (c k) n -> p c k n", c=CHG)
                v1 = up.tile([128, CHG, 2, 128], f16, tag="u1")
                v2 = up.tile([128, CHG, 2, 128], f16, tag="u2")
                v3 = up.tile([128, CHG, 2, 128], f16, tag="u3")
                v4 = up.tile([128, CHG, 2, 128], f16, tag="u4")
                nc.vector.tensor_mul(v1[:], r_re, kre_b)
                nc.vector.tensor_mul(v2[:], r_im, kim_b)
                nc.vector.tensor_sub(p_re, v1[:], v2[:])
                nc.vector.tensor_mul(v3[:], r_re, kim_b)
                nc.vector.tensor_mul(v4[:], r_im, kre_b)
                nc.vector.tensor_add(p_im, v3[:], v4[:])

            def e_i1(it):
                s = st[it]
                p_t = s["p"]
                s["csrc"] = wp.tile([128, 2, 2 * CHG, 128], f16, tag="csrc", name="csrc")
                for ch in range(CHG):
                    pc = pcp.tile([128, 2, 2, 128], f32, tag="pc")
                    for pk in range(2):
                        j = 2 * ch + pk
                        nc.tensor.matmul(
                            pc[:, :, pk, :], p_t[:, 0, j, :], fim[:, 0:256],
                            start=True, stop=False,
                        )
                        nc.tensor.matmul(
                            pc[:, :, pk, :], p_t[:, 1, j, :], fim[:, 256:512],
                            start=False, stop=True,
                        )
                    nc.scalar.copy(
                        out=s["csrc"][:, :, 2 * ch : 2 * ch + 2, :], in_=pc[:]
                    )

            def e_t2(it):
                s = st[it]
                c_re = s["csrc"][:, 0, :, :]
                c_im = s["csrc"][:, 1, :, :]
                cp_t = s["cp"] = wp.tile([128, 2, 2 * CHG, 128], f16, tag="cp", name="cp_t")
                w1 = up.tile([128, 2 * CHG, 128], f16, tag="u1")
                w2 = up.tile([128, 2 * CHG, 128], f16, tag="u2")
                w3 = up.tile([128, 2 * CHG, 128], f16, tag="u3")
                w4 = up.tile([128, 2 * CHG, 128], f16, tag="u4")
                nc.vector.tensor_mul(w1[:], c_re, t1re_b)
                nc.vector.tensor_mul(w2[:], c_im, t1im_b)
                nc.vector.tensor_mul(w3[:], c_re, t1im_b)
                nc.vector.tensor_mul(w4[:], c_im, t1re_b)
                nc.vector.tensor_add(cp_t[:, 0, :, :], w1[:], w2[:])
                nc.vector.tensor_sub(cp_t[:, 1, :, :], w4[:], w3[:])

            def e_i2(it):
                s = st[it]
                cp_t = s["cp"]
                py = None
                for ch in range(CHG):
                    cl = ch % 2
                    if cl == 0:
                        py = pyp.tile([128, 2, 2, 128], f32, tag="py")
                    for pk in range(2):
                        j = 2 * ch + pk
                        nc.tensor.matmul(
                            py[:, cl, pk, :], gcs[:], cp_t[:, 0, j, :],
                            start=True, stop=False,
                        )
                        nc.tensor.matmul(
                            py[:, cl, pk, :], gnc[:], cp_t[:, 1, j, :],
                            start=False, stop=True,
                        )
                    if cl == 1:
                        pair = (CHG * it + ch) // 2
                        ysb = op.tile([128, 2, 2, 128], f16, tag="ysb")
                        nc.scalar.copy(out=ysb[:], in_=py[:])
                        nc.sync.dma_start(
                            yw[pair].rearrange("c p k n -> p c k n"), ysb[:]
                        )
                del st[it]

            for pi in range(NIT // 2):
                e, o = 2 * pi, 2 * pi + 1
                e_dma(e)
                e_dma(o)
                e_f1(e)
                e_f1(o)
                e_t1(e)
                e_f2(e)
                e_t1(o)
                e_f2(o)
                e_pw(e)
                e_i1(e)
                e_pw(o)
                e_i1(o)
                e_t2(e)
                e_i2(e)
                e_t2(o)
                e_i2(o)

    nc.compile()
    return nc


def _host_arrays():
    cst = _consts()
    F_cos, F_sin = cst["F_cos"], cst["F_sin"]
    Tw_cos, Tw_sin = cst["Tw_cos"], cst["Tw_sin"]
    f16 = np.float16
    cosF, sinF = F_cos[:64, :], F_sin[:64, :]
    arrs = {}
    # stacked F1 moving: rows 0:64 act on z_re, rows 64:128 on z_im
    arrs["f1m"] = np.block([[cosF, -sinF], [sinF, cosF]]).astype(f16)
    arrs["f2c"] = F_cos.astype(f16)
    arrs["f2s"] = F_sin.astype(f16)
    arrs["f2sn"] = (-F_sin).astype(f16)
    arrs["fim"] = np.concatenate(
        [F_cos, F_sin, -F_sin, F_cos], axis=1
    ).astype(f16)
    # stacked I2 stationaries: out partitions 0:64 = Re (y even batch),
    # 64:128 = Im (y odd batch)
    arrs["gcs"] = np.concatenate(
        [F_cos[:, :64], F_sin[:, :64]], axis=1
    ).astype(f16)
    arrs["gnc"] = np.concatenate(
        [-F_sin[:, :64], F_cos[:, :64]], axis=1
    ).astype(f16)
    arrs["t1re"] = Tw_cos.astype(f16)
    arrs["t1im"] = (-Tw_sin).astype(f16)
    return arrs


def _prep_inputs(x, filt):
    """Full inputs -> list of per-core input maps."""
    consts = _host_arrays()

    kpad = np.zeros((D, NFFT), np.float64)
    kpad[:, :L] = filt
    Kf = (np.fft.fft(kpad, axis=1) / NFFT).reshape(D, 128, 128)  # [c, k2, k1]

    # x -> (D, 2pk, 2ri, 64 n1, 128 n2)
    xq = np.ascontiguousarray(x.transpose(2, 0, 1)).reshape(D, 2, 2, 64, 128)

    in_maps = []
    for ci in range(NC):
        sl = slice(ci * CPC, (ci + 1) * CPC)
        m = dict(consts)
        xc = xq[sl].reshape(NIT, CHG, 2, 2, 64, 128)
        # -> (it, (ri,n1), (chl,pk), n2)
        m["xw"] = np.ascontiguousarray(
            xc.transpose(0, 3, 4, 1, 2, 5).reshape(NIT, 128, 2 * CHG, 128)
        ).astype(np.float16)
        kc = Kf[sl]
        kri = np.stack([kc.real, kc.imag], axis=1)  # (CPC, 2ri, 128k2, 128k1)
        m["kk"] = np.ascontiguousarray(
            kri.reshape(NIT, CHG, 2, 128, 128).transpose(0, 3, 1, 2, 4)
        ).astype(np.float16)
        in_maps.append(m)
    return in_maps


def _post_outputs(res):
    y = np.empty((B, L, D), np.float32)
    for ci in range(NC):
        sl = slice(ci * CPC, (ci + 1) * CPC)
        # (pair, cl, (sig,n1), pk, n2); c = 2*pair+cl, b = 2*pk+sig,
        # l = 128*n1+n2
        r = res.results[ci]["yw"].astype(np.float32)
        r = r.reshape(CPC // 2, 2, 2, 64, 2, 128)
        r = r.transpose(4, 2, 3, 5, 0, 1).reshape(B, L, CPC)
        y[:, :, sl] = r
    return y


def kernel(x: np.ndarray, filt: np.ndarray) -> np.ndarray:
    from concourse.bass_utils import run_bass_kernel_spmd

    assert x.shape == (B, L, D) and filt.shape == (D, L)
    x = np.ascontiguousarray(x, dtype=np.float32)
    filt = np.ascontiguousarray(filt, dtype=np.float32)

    in_maps = _prep_inputs(x, filt)
    nc = _build_program()
    res = run_bass_kernel_spmd(nc, in_maps, core_ids=list(range(NC)))
    return _post_outputs(res)


def run_profiled(inputs):
    """Build + run with NTFF tracing; returns BassKernelResults (test-only)."""
    from concourse.bass_utils import run_bass_kernel_spmd

    x = np.ascontiguousarray(inputs["x"], dtype=np.float32)
    filt = np.ascontiguousarray(inputs["filt"], dtype=np.float32)
    in_maps = _prep_inputs(x, filt)
    nc = _build_program()
    return run_bass_kernel_spmd(
        nc, in_maps, core_ids=list(range(NC)), trace=True
    )


if __name__ == "__main__":
    rng = np.random.default_rng(0)
    x = rng.standard_normal((B, L, D)).astype(np.float32)
    filt = rng.standard_normal((D, L)).astype(np.float32)
    y = kernel(x, filt)
    print("y", y.shape, y.dtype, float(np.abs(y).max()))


# revision 11
# speedup vs baseline: 8.0572x; 1.1427x over previous
"""
LongConvolution (causal FFT conv) Trainium2 Bass kernel — v3.

Problem: x (4, 8192, 1024) f32, filt (1024, 8192) f32.
  y[b, l, c] = sum_m x[b, m, c] * filt[c, l - m]   (causal, per-channel)
Reference computes this via zero-padded FFT of size N = 16384 = 128*128.

Strategy
--------
1. Packed-complex batches: z = x[2p] + i*x[2p+1].  The filter multiply is
   linear, so IFFT(FFT(z) * K) = y[2p] + i*y[2p+1] with NO Hermitian
   unpacking.  4 real convs become 2 complex pipelines: halves F2/I1
   matmul work and ALL twiddle/pointwise elementwise work.
2. f16 everywhere on-chip: matmuls at 1 cycle/row, DVE elementwise at
   2x rate.  PSUM stays f32.
3. All 18 elementwise ops on DVE.  (v2 offloaded 4 to GpSimd: SBUF port
   contention stretched concurrent DVE ops to GpSimd's duration — net
   loss.  ACT does only PSUM->SBUF converting copies.)
4. Stacked stationaries kill matmuls+LDWEIGHTS: F1 contracts the
   stacked [z_re; z_im] (K=128, one matmul per packed signal); I2 uses
   [gc|gs] / [-gs|gc] so Re and Im outputs come from one matmul pair,
   with two channels sharing a PSUM bank along the free dim.
5. 4-channel iterations: elementwise free-size 1024; per-channel PSUM
   tiles (1 bank) so all stages double-buffer in 8 banks.

Four-step FFT (k = 128*k2 + k1, n = 128*n1 + n2):
  A[n2,k1] = sum_n1 e^{-2pi i n1 k1/128} z[128 n1 + n2]     (F1, K=64x2)
  B = A * T1,  T1[n2,k1] = e^{-2pi i n2 k1/N}               (T1, DVE)
  R[k2,k1] = sum_n2 e^{-2pi i n2 k2/128} B[n2,k1]           (F2)
  P = R * K   (filter spectrum, host-precomputed, [k2,k1])  (PW)
  C[k1,n2] = sum_k2 e^{+2pi i n2 k2/128} P[k2,k1]           (I1)
  C' = C * conj(T1)  ([k1,n2] layout, T1 symmetric)         (T2)
  y[n1,n2] = sum_k1 e^{+2pi i n1 k1/128} C'[k1,n2], n1<64   (I2)
  y[2p] = Re, y[2p+1] = Im.

Sharding: d_model across the 8 cores (128 channels each).
"""

import os
import sys

import numpy as np

for p in ("/opt/trn_rl_repo",):
    if p not in sys.path:
        sys.path.insert(0, p)

os.environ.setdefault("MYCRO_LOCAL_CACHE", "1")

# ----------------------------------------------------------------------------
# configuration
# ----------------------------------------------------------------------------
B, L, D = 4, 8192, 1024
NFFT = 2 * L               # 16384 = 128 * 128
NC = 8                     # cores
CPC = D // NC              # channels per core = 128
CHG = 8                    # channels per iteration
NIT = CPC // CHG           # 32 iterations


def _consts():
    j = np.arange(128)
    ang128 = 2 * np.pi * np.outer(j, j) / 128
    angN = 2 * np.pi * np.outer(j, j) / NFFT
    return {
        "F_cos": np.cos(ang128), "F_sin": np.sin(ang128),
        "Tw_cos": np.cos(angN), "Tw_sin": np.sin(angN),
    }


def _build_program():
    import concourse.bacc as bacc
    import concourse.mybir as mybir
    from concourse import tile

    f32 = mybir.dt.float32
    f16 = mybir.dt.float16

    nc = bacc.Bacc(None, target_bir_lowering=False, debug=False)

    # --- DRAM I/O (all f16) ---
    # xw[it, (ri,n1), 2*chl+pk, n2] — stacked z_re/z_im chunks
    xw = nc.dram_tensor("xw", (NIT, 128, 2 * CHG, 128), f16, kind="ExternalInput")
    # kk[it, k2, chl, ri, k1]
    kk = nc.dram_tensor("kk", (NIT, 128, CHG, 2, 128), f16, kind="ExternalInput")
    f1m_d = nc.dram_tensor("f1m", (128, 256), f16, kind="ExternalInput")
    f2c_d = nc.dram_tensor("f2c", (128, 128), f16, kind="ExternalInput")
    f2s_d = nc.dram_tensor("f2s", (128, 128), f16, kind="ExternalInput")
    f2sn_d = nc.dram_tensor("f2sn", (128, 128), f16, kind="ExternalInput")
    fim_d = nc.dram_tensor("fim", (128, 512), f16, kind="ExternalInput")
    gcs_d = nc.dram_tensor("gcs", (128, 128), f16, kind="ExternalInput")
    gnc_d = nc.dram_tensor("gnc", (128, 128), f16, kind="ExternalInput")
    t1re_d = nc.dram_tensor("t1re", (128, 2, 128), f16, kind="ExternalInput")
    t1im_d = nc.dram_tensor("t1im", (128, 2, 128), f16, kind="ExternalInput")
    gncn_d = nc.dram_tensor("gncn", (128, 128), f16, kind="ExternalInput")
    # yw[pair, cl, (sig,n1), pk, n2] — two channels share one PSUM bank
    yw = nc.dram_tensor(
        "yw", (CPC // 2, 2, 128, 2, 128), f16, kind="ExternalOutput"
    )

    with tile.TileContext(nc) as tc:
        with (
            tc.tile_pool(name="const", bufs=1) as constp,
            tc.tile_pool(name="m", bufs=2) as mp,
            tc.tile_pool(name="kf", bufs=2) as kp,
            tc.tile_pool(name="work", bufs=2) as wp,
            tc.tile_pool(name="scr", bufs=2) as up,
            tc.tile_pool(name="out", bufs=3) as op,
            tc.tile_pool(name="pa", bufs=2, space="PSUM") as pap,
            tc.tile_pool(name="pr", bufs=2, space="PSUM") as prp,
            tc.tile_pool(name="pc", bufs=2, space="PSUM") as pcp,
            tc.tile_pool(name="py", bufs=2, space="PSUM") as pyp,
        ):
            f1m = constp.tile([128, 256], f16)
            f2c = constp.tile([128, 128], f16)
            f2s = constp.tile([128, 128], f16)
            f2sn = constp.tile([128, 128], f16)
            fim = constp.tile([128, 512], f16)
            gcs = constp.tile([128, 128], f16)
            gnc = constp.tile([128, 128], f16)
            gncn = constp.tile([128, 128], f16)
            t1re = constp.tile([128, 2, 128], f16)
            t1im = constp.tile([128, 2, 128], f16)
            for t, d in (
                (f1m, f1m_d), (f2c, f2c_d), (f2s, f2s_d), (f2sn, f2sn_d),
                (fim, fim_d), (gcs, gcs_d), (gnc, gnc_d), (gncn, gncn_d),
                (t1re, t1re_d), (t1im, t1im_d),
            ):
                nc.sync.dma_start(t[:], d[:])
            t1re_b = (
                t1re[:].rearrange("p r (s n) -> p r s n", s=1)
                .broadcast_to([128, 2, 2 * CHG, 128])
            )
            t1im_b = (
                t1im[:].rearrange("p r (s n) -> p r s n", s=1)
                .broadcast_to([128, 2, 2 * CHG, 128])
            )

            # Per-iteration stage emitters.  Stages of iteration pairs are
            # emitted interleaved (2-wide software pipelining) so every
            # engine's in-order stream has independent work between
            # dependent stages of one iteration.
            st = {}  # it -> dict of live tiles

            def e_dma(it):
                s = st[it] = {}
                s["m4"] = mp.tile([128, 2 * CHG, 128], f16, tag="m", name="m4")
                nc.sync.dma_start(s["m4"][:], xw[it])
                s["kt"] = kp.tile([128, CHG, 2, 128], f16, tag="k", name="kt")
                nc.sync.dma_start(s["kt"][:], kk[it])

            def e_f1(it):
                s = st[it]
                s["asrc"] = wp.tile([128, 2, 2 * CHG, 128], f16, tag="asrc", name="asrc")
                for ch in range(CHG):
                    pa = pap.tile([128, 2, 2, 128], f32, tag="pa")
                    for pk in range(2):
                        nc.tensor.matmul(
                            pa[:, :, pk, :], s["m4"][:, 2 * ch + pk, :],
                            f1m[:], start=True, stop=True,
                        )
                    nc.scalar.copy(
                        out=s["asrc"][:, :, 2 * ch : 2 * ch + 2, :], in_=pa[:]
                    )

            def e_t1(it):
                s = st[it]
                b_t = s["b"] = wp.tile([128, 2, 2 * CHG, 128], f16, tag="b", name="b_t")
                u14 = up.tile([128, 2, 2 * CHG, 128], f16, tag="u14", name="u14")
                u23 = up.tile([128, 2, 2 * CHG, 128], f16, tag="u23", name="u23")
                # u14 = [a_re; a_im] * t1re ; u23 = [a_re; a_im] * t1im
                nc.vector.tensor_mul(u14[:], s["asrc"][:], t1re_b)
                nc.vector.tensor_mul(u23[:], s["asrc"][:], t1im_b)
                nc.vector.tensor_sub(
                    b_t[:, 0, :, :], u14[:, 0, :, :], u23[:, 1, :, :]
                )
                nc.vector.tensor_add(
                    b_t[:, 1, :, :], u23[:, 0, :, :], u14[:, 1, :, :]
                )

            def e_f2(it):
                s = st[it]
                b_t = s["b"]
                s["rsrc"] = wp.tile([128, 2, 2 * CHG, 128], f16, tag="rsrc", name="rsrc")
                for ch in range(CHG):
                    pr = prp.tile([128, 2, 2, 128], f32, tag="pr")
                    for pk in range(2):
                        j = 2 * ch + pk
                        b_re = b_t[:, 0, j, :]
                        b_im = b_t[:, 1, j, :]
                        nc.tensor.matmul(
                            pr[:, 0, pk, :], f2c[:], b_re,
                            start=True, stop=False,
                        )
                        nc.tensor.matmul(
                            pr[:, 0, pk, :], f2s[:], b_im,
                            start=False, stop=True,
                        )
                        nc.tensor.matmul(
                            pr[:, 1, pk, :], f2c[:], b_im,
                            start=True, stop=False,
                        )
                        nc.tensor.matmul(
                            pr[:, 1, pk, :], f2sn[:], b_re,
                            start=False, stop=True,
                        )
                    nc.scalar.copy(
                        out=s["rsrc"][:, :, 2 * ch : 2 * ch + 2, :], in_=pr[:]
                    )

            def e_pw(it):
                s = st[it]
                rsrc, kt = s["rsrc"], s["kt"]
                r_re = rsrc[:, 0, :, :].rearrange("p (c k) n -> p c k n", c=CHG)
                r_im = rsrc[:, 1, :, :].rearrange("p (c k) n -> p c k n", c=CHG)
                kre_b = (
                    kt[:, :, 0, :].rearrange("p c (s n) -> p c s n", s=1)
                    .broadcast_to([128, CHG, 2, 128])
                )
                kim_b = (
                    kt[:, :, 1, :].rearrange("p c (s n) -> p c s n", s=1)
                    .broadcast_to([128, CHG, 2, 128])
                )
                p_t = s["p"] = wp.tile([128, 2, 2 * CHG, 128], f16, tag="p", name="p_t")
                p_re = p_t[:, 0, :, :].rearrange("p (c k) n -> p c k n", c=CHG)
                p_im = p_t[:, 1, :, :].rearrange("p (c k) n -> p c k n", c=CHG)
                v1 = up.tile([128, CHG, 2, 128], f16, tag="u1")
                v2 = up.tile([128, CHG, 2, 128], f16, tag="u2")
                v3 = up.tile([128, CHG, 2, 128], f16, tag="u3")
                v4 = up.tile([128, CHG, 2, 128], f16, tag="u4")
                nc.vector.tensor_mul(v1[:], r_re, kre_b)
                nc.vector.tensor_mul(v2[:], r_im, kim_b)
                nc.vector.tensor_sub(p_re, v1[:], v2[:])
                nc.vector.tensor_mul(v3[:], r_re, kim_b)
                nc.vector.tensor_mul(v4[:], r_im, kre_b)
                nc.vector.tensor_add(p_im, v3[:], v4[:])

            def e_i1(it):
                s = st[it]
                p_t = s["p"]
                s["csrc"] = wp.tile([128, 2, 2 * CHG, 128], f16, tag="csrc", name="csrc")
                for ch in range(CHG):
                    pc = pcp.tile([128, 2, 2, 128], f32, tag="pc")
                    for pk in range(2):
                        j = 2 * ch + pk
                        nc.tensor.matmul(
                            pc[:, :, pk, :], p_t[:, 0, j, :], fim[:, 0:256],
                            start=True, stop=False,
                        )
                        nc.tensor.matmul(
                            pc[:, :, pk, :], p_t[:, 1, j, :], fim[:, 256:512],
                            start=False, stop=True,
                        )
                    nc.scalar.copy(
                        out=s["csrc"][:, :, 2 * ch : 2 * ch + 2, :], in_=pc[:]
                    )

            def e_t2(it):
                # w14 = [c_re; c_im] * t1re -> (w1, w4)
                # w23 = [c_re; c_im] * t1im -> (w3, w2)
                # cp_re = w1 + w2, cp_im = w4 - w3: folded into I2 matmuls
                s = st[it]
                w14 = s["w14"] = wp.tile(
                    [128, 2, 2 * CHG, 128], f16, tag="w14", name="w14"
                )
                w23 = s["w23"] = wp.tile(
                    [128, 2, 2 * CHG, 128], f16, tag="w23", name="w23"
                )
                nc.vector.tensor_mul(w14[:], s["csrc"][:], t1re_b)
                nc.vector.tensor_mul(w23[:], s["csrc"][:], t1im_b)

            def e_i2(it):
                s = st[it]
                w14, w23 = s["w14"], s["w23"]
                py = None
                for ch in range(CHG):
                    cl = ch % 2
                    if cl == 0:
                        py = pyp.tile([128, 2, 2, 128], f32, tag="py")
                    for pk in range(2):
                        j = 2 * ch + pk
                        # y = gcs@(w1+w2) + gnc@(w4-w3)
                        nc.tensor.matmul(
                            py[:, cl, pk, :], gcs[:], w14[:, 0, j, :],
                            start=True, stop=False,
                        )
                        nc.tensor.matmul(
                            py[:, cl, pk, :], gcs[:], w23[:, 1, j, :],
                            start=False, stop=False,
                        )
                        nc.tensor.matmul(
                            py[:, cl, pk, :], gnc[:], w14[:, 1, j, :],
                            start=False, stop=False,
                        )
                        nc.tensor.matmul(
                            py[:, cl, pk, :], gncn[:], w23[:, 0, j, :],
                            start=False, stop=True,
                        )
                    if cl == 1:
                        pair = (CHG * it + ch) // 2
                        ysb = op.tile([128, 2, 2, 128], f16, tag="ysb")
                        nc.scalar.copy(out=ysb[:], in_=py[:])
                        nc.sync.dma_start(
                            yw[pair].rearrange("c p k n -> p c k n"), ysb[:]
                        )
                del st[it]

            for pi in range(NIT // 2):
                e, o = 2 * pi, 2 * pi + 1
                e_dma(e)
                e_dma(o)
                e_f1(e)
                e_f1(o)
                e_t1(e)
                e_f2(e)
                e_t1(o)
                e_f2(o)
                e_pw(e)
                e_i1(e)
                e_pw(o)
                e_i1(o)
                e_t2(e)
                e_i2(e)
                e_t2(o)
                e_i2(o)

    nc.compile()
    return nc


def _host_arrays():
    cst = _consts()
    F_cos, F_sin = cst["F_cos"], cst["F_sin"]
    Tw_cos, Tw_sin = cst["Tw_cos"], cst["Tw_sin"]
    f16 = np.float16
    cosF, sinF = F_cos[:64, :], F_sin[:64, :]
    arrs = {}
    # stacked F1 moving: rows 0:64 act on z_re, rows 64:128 on z_im
    arrs["f1m"] = np.block([[cosF, -sinF], [sinF, cosF]]).astype(f16)
    arrs["f2c"] = F_cos.astype(f16)
    arrs["f2s"] = F_sin.astype(f16)
    arrs["f2sn"] = (-F_sin).astype(f16)
    arrs["fim"] = np.concatenate(
        [F_cos, F_sin, -F_sin, F_cos], axis=1
    ).astype(f16)
    # stacked I2 stationaries: out partitions 0:64 = Re (y even batch),
    # 64:128 = Im (y odd batch)
    arrs["gcs"] = np.concatenate(
        [F_cos[:, :64], F_sin[:, :64]], axis=1
    ).astype(f16)
    arrs["gnc"] = np.concatenate(
        [-F_sin[:, :64], F_cos[:, :64]], axis=1
    ).astype(f16)
    arrs["gncn"] = np.concatenate(
        [F_sin[:, :64], -F_cos[:, :64]], axis=1
    ).astype(f16)
    arrs["t1re"] = np.stack([Tw_cos, Tw_cos], axis=1).astype(f16)
    arrs["t1im"] = np.stack([-Tw_sin, -Tw_sin], axis=1).astype(f16)
    return arrs


def _prep_inputs(x, filt):
    """Full inputs -> list of per-core input maps."""
    consts = _host_arrays()

    kpad = np.zeros((D, NFFT), np.float64)
    kpad[:, :L] = filt
    Kf = (np.fft.fft(kpad, axis=1) / NFFT).reshape(D, 128, 128)  # [c, k2, k1]

    # x -> (D, 2pk, 2ri, 64 n1, 128 n2)
    xq = np.ascontiguousarray(x.transpose(2, 0, 1)).reshape(D, 2, 2, 64, 128)

    in_maps = []
    for ci in range(NC):
        sl = slice(ci * CPC, (ci + 1) * CPC)
        m = dict(consts)
        xc = xq[sl].reshape(NIT, CHG, 2, 2, 64, 128)
        # -> (it, (ri,n1), (chl,pk), n2)
        m["xw"] = np.ascontiguousarray(
            xc.transpose(0, 3, 4, 1, 2, 5).reshape(NIT, 128, 2 * CHG, 128)
        ).astype(np.float16)
        kc = Kf[sl]
        kri = np.stack([kc.real, kc.imag], axis=1)  # (CPC, 2ri, 128k2, 128k1)
        m["kk"] = np.ascontiguousarray(
            kri.reshape(NIT, CHG, 2, 128, 128).transpose(0, 3, 1, 2, 4)
        ).astype(np.float16)
        in_maps.append(m)
    return in_maps


def _post_outputs(res):
    y = np.empty((B, L, D), np.float32)
    for ci in range(NC):
        sl = slice(ci * CPC, (ci + 1) * CPC)
        # (pair, cl, (sig,n1), pk, n2); c = 2*pair+cl, b = 2*pk+sig,
        # l = 128*n1+n2
        r = res.results[ci]["yw"].astype(np.float32)
        r = r.reshape(CPC // 2, 2, 2, 64, 2, 128)
        r = r.transpose(4, 2, 3, 5, 0, 1).reshape(B, L, CPC)
        y[:, :, sl] = r
    return y


def kernel(x: np.ndarray, filt: np.ndarray) -> np.ndarray:
    from concourse.bass_utils import run_bass_kernel_spmd

    assert x.shape == (B, L, D) and filt.shape == (D, L)
    x = np.ascontiguousarray(x, dtype=np.float32)
    filt = np.ascontiguousarray(filt, dtype=np.float32)

    in_maps = _prep_inputs(x, filt)
    nc = _build_program()
    res = run_bass_kernel_spmd(nc, in_maps, core_ids=list(range(NC)))
    return _post_outputs(res)


def run_profiled(inputs):
    """Build + run with NTFF tracing; returns BassKernelResults (test-only)."""
    from concourse.bass_utils import run_bass_kernel_spmd

    x = np.ascontiguousarray(inputs["x"], dtype=np.float32)
    filt = np.ascontiguousarray(inputs["filt"], dtype=np.float32)
    in_maps = _prep_inputs(x, filt)
    nc = _build_program()
    return run_bass_kernel_spmd(
        nc, in_maps, core_ids=list(range(NC)), trace=True
    )


if __name__ == "__main__":
    rng = np.random.default_rng(0)
    x = rng.standard_normal((B, L, D)).astype(np.float32)
    filt = rng.standard_normal((D, L)).astype(np.float32)
    y = kernel(x, filt)
    print("y", y.shape, y.dtype, float(np.abs(y).max()))
